# revision 12
# baseline (speedup 1.0000x reference)
"""Trainium2 Bass kernel for a 12-head MHA layer with relative position bias
and a 0/1 attention mask (B=2, N=2048, C=768, H=12, d=64), sharded over 8
NeuronCores (batch x head-group parallel: core c handles batch c//4 and heads
3*(c%4) .. 3*(c%4)+2).

v3: phase D runs PAIRED k-tile steps: two ST matmuls execute concurrently on
row-groups (0,0)/(64,0) of the PE array (head h0/h2even lives at partitions
0-63, h1/h2odd at 64-127; the odd head's q/k weights are duplicated into the
pad half-columns so both row groups stay balanced 24:24). Each paired step
produces one [128, 1024] PSUM tile = [ST_A 512 | ST_B 512] over a q-quarter,
consumed by either:
  - ACT path: exp -> bf16, then DVE mult with a bf16 exp(bias)*mask pair tile
  - fused DVE path (1/4 of steps): e1_bits = int16(S*A16 + T) in one
    scalar_tensor_tensor, T = round(A16*bias + (B16-C)) with mask folded as a
    -25000 sentinel (bitcasts to ~-5e-9); e1 = bitcast_bf16(e1_bits)
The eb stream is host-packed in (quarter, step) schedule order so each step
is one DMA. PV accumulates per (head, quarter) into 1-bank [66,512] tiles
(row 64 = softmax denominator via the ones-column in v').

Junk warm-keeper matmuls cover the E-phase ln/exp window so the final
projection runs with the PE clock still hot.
"""

import os
import numpy as np
import ml_dtypes

import concourse.bass as bass
import concourse.tile as tile
from concourse.tile import add_dep_helper
from concourse import bacc, mybir
from concourse.alu_op_type import AluOpType
from concourse.bass_utils import run_bass_kernel_spmd

AF = mybir.ActivationFunctionType
DT = mybir.dt
F32R = mybir.dt.float32r

B, N, C, H, D = 2, 2048, 768, 12, 64
HPC = H // 4          # heads per core (8 cores = 2 batches x 4 head-groups)
NCORES = 8
SCALE = float(D) ** -0.5

# ---- fused integer-exp (Schraudolph, int16/bf16-bitcast form) ----
A16 = (1 << 7) / np.log(2.0)          # 184.66496...
B16 = 127 << 7                        # 16256
C16 = 5.1                             # spline-center correction (HW-tuned)
T_MASKED = -25000                     # int16 sentinel -> bf16 ~ -5e-9

QTR = 512                             # q-quarter width
NQTR = N // QTR

LAST_RESULTS = None   # BassKernelResults of the most recent kernel() call


def pair_schedule(hpc=HPC, nt=16):
    """Paired step schedule: Q0 runs on PE row-group 0 (partition base 0),
    Q1 on row-group 64. Heads i%2==0 live at base 0, i%2==1 at base 64; the
    odd head (h2) is split even-j/odd-j across the bases via weight dup.
    Returns (steps, sch_steps): steps[s] = ((headA, jA), (headB, jB))."""
    assert hpc == 3
    q0 = [(0, j) for j in range(nt)] + [(2, j) for j in range(0, nt, 2)]
    q1 = [(1, j) for j in range(nt)] + [(2, j) for j in range(1, nt, 2)]
    steps = list(zip(q0, q1))
    sch = tuple(s for s in range(len(steps)) if s % 4 == 2)  # 6/24 = 1/4
    return steps, sch


def build_program(n=N, c_in=C, hpc=HPC, d=D, c_out=C):
    """Build the per-core Bass/Tile program. Same program runs on all cores
    (SPMD); per-core data differs via in_maps."""
    nt = n // 128                       # number of 128-row k-tiles
    qch = _q_chunks(n)
    ck = (c_in + 127) // 128            # contraction chunks over c_in
    n_qk_chunks = 2 * ((hpc + 1) // 2)  # 4 for hpc=3
    wqk_cols = 128 * n_qk_chunks
    wv_cols = hpc * (d + 2)             # [v_i | ones | pad] per head
    mo = c_out // 128                   # proj output row chunks

    steps, sch_steps = pair_schedule(hpc, nt)
    nsteps = len(steps)
    act_steps = tuple(s for s in range(nsteps) if s not in sch_steps)
    nact, nsch = len(act_steps), len(sch_steps)

    def pc(kc):
        return min(128, c_in - 128 * kc)

    nc = bacc.Bacc("TRN2", target_bir_lowering=False, debug=False)
    xt = nc.dram_tensor("xt", [c_in, n], DT.bfloat16, kind="ExternalInput").ap()
    wqk = nc.dram_tensor("wqk", [c_in, wqk_cols], DT.bfloat16, kind="ExternalInput").ap()
    wv = nc.dram_tensor("wv", [c_in, wv_cols], DT.bfloat16, kind="ExternalInput").ap()
    ebb = nc.dram_tensor("ebb", [NQTR, nact, 128, 2 * QTR], DT.bfloat16,
                         kind="ExternalInput").ap()
    ebi = nc.dram_tensor("ebi", [NQTR, nsch, 128, 2 * QTR], DT.int16,
                         kind="ExternalInput").ap()
    pw = nc.dram_tensor("pw", [hpc * d, c_out], DT.bfloat16, kind="ExternalInput").ap()
    yt = nc.dram_tensor("yt", [c_out, n], DT.float32, kind="ExternalOutput").ap()

    with tile.TileContext(nc) as tc:
        # ---- pools (stack allocator: xts/weights released before attn) ----
        persist = tc.alloc_tile_pool(name="persist", bufs=1)
        qkvout = tc.alloc_tile_pool(name="qkvout", bufs=1)
        loadp = tc.alloc_tile_pool(name="loadp", bufs=1)
        ps_qkv = tc.alloc_tile_pool(name="ps_qkv", bufs=4, space="PSUM")

        pw_s = persist.tile([64, hpc, c_out], DT.bfloat16, tag="pw")
        ones_s = persist.tile([1, 128], DT.float32, tag="ones")
        nc.vector.memset(ones_s, 1.0)
        ones3 = persist.tile([128, hpc], DT.float32, tag="ones3")
        nc.vector.memset(ones3, 1.0)
        ones_r = persist.tile([1, 128], F32R, tag="ones_r")
        nc.vector.tensor_copy(ones_r, ones_s)
        for i in range(hpc):
            nc.gpsimd.dma_start(out=pw_s[:, i, :], in_=pw[64 * i:64 * i + 64, :])

        qk_s = qkvout.tile([128, n_qk_chunks, n], DT.bfloat16, tag="qk")
        v_s = qkvout.tile([128, nt, wv_cols], DT.bfloat16, tag="v")

        xts = loadp.tile([128, ck, n], DT.bfloat16, tag="xts")
        wqk_s = loadp.tile([128, ck, wqk_cols], DT.bfloat16, tag="wqk")
        wv_s = loadp.tile([128, ck, wv_cols], DT.bfloat16, tag="wv")
        for kc in range(ck):
            p = pc(kc)
            nc.gpsimd.dma_start(out=xts[:p, kc, :], in_=xt[128 * kc:128 * kc + p, :])
            nc.gpsimd.dma_start(out=wqk_s[:p, kc, :], in_=wqk[128 * kc:128 * kc + p, :])
            nc.gpsimd.dma_start(out=wv_s[:p, kc, :], in_=wv[128 * kc:128 * kc + p, :])

        # ---- phase B: qkT = wqk.T @ xT  -> qk_s ----
        def emit_qk_chunk(m):
            for (fo, fs) in qch:
                ps = ps_qkv.tile([128, 512], DT.float32, tag="psqkv", name=f"psB{m}")
                for kc in range(ck):
                    p = pc(kc)
                    nc.tensor.matmul(
                        ps[:, :fs],
                        lhsT=wqk_s[:p, kc, 128 * m:128 * m + 128],
                        rhs=xts[:p, kc, fo:fo + fs],
                        start=(kc == 0), stop=(kc == ck - 1),
                    )
                nc.vector.tensor_copy(qk_s[:, m, fo:fo + fs], ps[:, :fs])

        for m in range(min(2, n_qk_chunks)):
            emit_qk_chunk(m)

        # ---- phase C: v' = xT.T @ wv -> v_s (natural layout, k on partitions)
        for j in range(nt):
            ps = ps_qkv.tile([128, wv_cols], DT.float32, tag="psqkv")
            for kc in range(ck):
                p = pc(kc)
                nc.tensor.matmul(
                    ps,
                    lhsT=xts[:p, kc, 128 * j:128 * j + 128],
                    rhs=wv_s[:p, kc, :],
                    start=(kc == 0), stop=(kc == ck - 1),
                )
            nc.vector.tensor_copy(v_s[:, j, :], ps)
            nc.vector.tensor_copy(
                v_s[:, j, :].rearrange("p (h c) -> p h c", c=d + 2)[:, :, d],
                ones3)

        for m in range(min(2, n_qk_chunks), n_qk_chunks):
            emit_qk_chunk(m)

        loadp.release()   # free xts/wqk_s/wv_s space for attention pools
        ps_qkv.release()  # free PSUM banks for the attention pools

        # ---- attention pools ----
        ebpB = tc.alloc_tile_pool(name="ebpB", bufs=8)
        ebpI = tc.alloc_tile_pool(name="ebpI", bufs=4)
        e1pool = tc.alloc_tile_pool(name="e1pool", bufs=4)
        normp = tc.alloc_tile_pool(name="normp", bufs=1)
        ps_st = tc.alloc_tile_pool(name="ps_st", bufs=3, space="PSUM")
        ps_ot = tc.alloc_tile_pool(name="ps_ot", bufs=2, space="PSUM")

        osum_all = normp.tile([66, hpc, n], DT.bfloat16, tag="osum")
        osum = [osum_all[:, i, :] for i in range(hpc)]

        def unit_aps(head, j):
            """(q, kv, base) for a (head, j) unit in the paired layout."""
            if head < 2:
                base = 64 * head
                qv = qk_s[base:base + 64, 2 * (head // 2), :]
                kv = qk_s[base:base + 64, 2 * (head // 2) + 1, :]
            else:
                base = 64 * (j % 2)   # h2: even j at base 0, odd j at base 64
                qv = qk_s[base:base + 64, 2, :]
                kv = qk_s[base:base + 64, 3, :]
            return qv, kv

        # ---- phase D: paired attention steps over q-quarters ----
        last_exp = None
        for qtr in range(NQTR):
            qo = qtr * QTR
            ots = {}
            for s, ((hA, jA), (hB, jB)) in enumerate(steps):
                # eb pair tile for this step (one DMA, host-packed)
                if s in sch_steps:
                    si = sch_steps.index(s)
                    ebt = ebpI.tile([128, 2 * QTR], DT.int16, tag="ebT",
                                    name="ebT")
                    nc.gpsimd.dma_start(out=ebt, in_=ebi[qtr, si])
                else:
                    ai = act_steps.index(s)
                    ebt = ebpB.tile([128, 2 * QTR], DT.bfloat16, tag="ebB",
                                    name="ebB")
                    nc.gpsimd.dma_start(out=ebt, in_=ebb[qtr, ai])

                st = ps_st.tile([128, 2 * QTR], DT.float32, tag="st")
                for half, (h, j) in enumerate(((hA, jA), (hB, jB))):
                    qv, kv = unit_aps(h, j)
                    nc.tensor.matmul(
                        st[:, half * QTR:(half + 1) * QTR],
                        lhsT=kv[:, 128 * j:128 * j + 128],
                        rhs=qv[:, qo:qo + QTR],
                        start=True, stop=True,
                    )

                if s in sch_steps:
                    e1i = e1pool.tile([128, 2 * QTR], DT.int16, tag="e1i",
                                      name="e1i")
                    nc.vector.scalar_tensor_tensor(
                        e1i, st, float(A16), ebt,
                        AluOpType.mult, AluOpType.add)
                    e1x = e1i.bitcast(DT.bfloat16)
                else:
                    e0 = e1pool.tile([128, 2 * QTR], DT.bfloat16, tag="e0",
                                     name="e0")
                    last_exp = nc.scalar.activation(e0, st, AF.Exp)
                    e1 = e1pool.tile([128, 2 * QTR], DT.bfloat16, tag="e1",
                                     name="e1")
                    nc.vector.tensor_tensor(e1, e0, ebt, AluOpType.mult)
                    e1x = e1

                for half, (h, j) in enumerate(((hA, jA), (hB, jB))):
                    if h not in ots:
                        ots[h] = (ps_ot.tile([66, QTR], DT.float32, tag="ot",
                                             name=f"ot{h}"), [0])
                    ot, cnt = ots[h]
                    nc.tensor.matmul(
                        ot,
                        lhsT=v_s[:, j, (d + 2) * h:(d + 2) * h + d + 2],
                        rhs=e1x[:, half * QTR:(half + 1) * QTR],
                        start=(cnt[0] == 0), stop=(cnt[0] == nt - 1),
                    )
                    cnt[0] += 1
                    if cnt[0] == nt:
                        nc.vector.tensor_copy(osum[h][:, qo:qo + QTR], ot)
                        del ots[h]

        # ---- phase E: normalization (deferred; recip = exp(-ln(sum))) ----
        # single ln + exp over all heads' denominator rows at once
        lnr_all = normp.tile([1, hpc * n], DT.float32, tag="lnr", name="lnr")
        rrow_all = normp.tile([1, hpc * n], F32R, tag="rrow", name="rrow")
        ln_inst = nc.scalar.activation(
            lnr_all, osum_all[64:65, :, :].rearrange("p a b -> p (a b)"), AF.Ln)
        if last_exp is not None:
            add_dep_helper(ln_inst.ins, last_exp.ins, sync=False,
                           reason="act-table ordering")
        nc.scalar.activation(rrow_all, lnr_all, AF.Exp, scale=-1.0)
        rrow3 = rrow_all.rearrange("p (a b) -> p a b", b=n)

        # junk warm-keeper matmuls: keep the PE active through the ln/exp
        # window (incl. 2 ACT table loads) so HAM stays at full clock for
        # the rps + F matmuls
        junk = ps_st.tile([64, 2 * QTR], DT.float32, tag="st", name="junk")
        for w in range(60):
            nc.tensor.matmul(
                junk[:, 0:512],
                lhsT=pw_s[:, w % hpc, 0:64],
                rhs=qk_s[0:64, 0, 0:512],
                start=True, stop=True,
            )

        for i in range(hpc):
            for h2 in range(2):
                ho = h2 * (n // 2)
                rps = ps_st.tile([64, n // 2], DT.float32, tag="st", name="rps")
                for (fo, fs) in _q_chunks(n // 2):
                    nc.tensor.matmul(
                        rps[:, fo:fo + fs],
                        lhsT=ones_r[0:1, 0:64],
                        rhs=rrow3[:, i, ho + fo:ho + fo + fs],
                        start=True, stop=True,
                    )
                # osum[0:64] *= 1/denom, fused from PSUM (in-place on in1)
                nc.vector.scalar_tensor_tensor(
                    osum[i][0:64, ho:ho + n // 2], rps, 1.0,
                    osum[i][0:64, ho:ho + n // 2],
                    AluOpType.mult, AluOpType.mult)

        # second junk batch: cover the DVE apply tail so F starts warm
        junk2 = ps_st.tile([64, 2 * QTR], DT.float32, tag="st", name="junk2")
        for w in range(16):
            nc.tensor.matmul(
                junk2[:, 0:512],
                lhsT=pw_s[:, w % hpc, 0:64],
                rhs=qk_s[0:64, 0, 0:512],
                start=True, stop=True,
            )

        ps_ot.release()
        ps_st.release()

        # ---- phase F: partial projection ytT = pw.T @ onrm ----
        ps_pj = tc.alloc_tile_pool(name="ps_pj", bufs=2, space="PSUM")
        ytp = tc.alloc_tile_pool(name="ytp", bufs=2)
        for m in range(mo):
            ps = ps_pj.tile([128, n], DT.float32, tag="pj")
            for (fo, fs) in qch:
                for i in range(hpc):
                    nc.tensor.matmul(
                        ps[:, fo:fo + fs],
                        lhsT=pw_s[:, i, 128 * m:128 * m + 128],
                        rhs=osum[i][0:64, fo:fo + fs],
                        start=(i == 0), stop=(i == hpc - 1),
                    )
            yts = ytp.tile([128, n], DT.float32, tag="yts")
            nc.scalar.copy(yts, ps)
            nc.sync.dma_start(out=yt[128 * m:128 * m + 128, :], in_=yts)

        ps_pj.release()
        ytp.release()
        normp.release()
        e1pool.release()
        ebpI.release()
        ebpB.release()
        qkvout.release()
        persist.release()

    nc.compile()
    return nc


def _q_chunks(n, c=512):
    out = []
    o = 0
    while o < n:
        sz = min(c, n - o)
        out.append((o, sz))
        o += sz
    return out


_PROG = {}


def _get_program(**kw):
    key = tuple(sorted(kw.items()))
    if key not in _PROG:
        _PROG[key] = build_program(**kw)
    return _PROG[key]


def make_in_maps(x, mask, qkv_w, qkv_b, rel_bias, proj_w):
    """Host-side shard + layout prep. Returns list of per-core input dicts."""
    x = np.asarray(x, dtype=np.float32)
    mask = np.asarray(mask)
    qkv_w = np.asarray(qkv_w, dtype=np.float32)
    qkv_b = np.asarray(qkv_b, dtype=np.float32)
    rel_bias = np.asarray(rel_bias, dtype=np.float32)
    proj_w = np.asarray(proj_w, dtype=np.float32)

    n_qk_chunks = 2 * ((HPC + 1) // 2)
    wqk_cols = 128 * n_qk_chunks
    wv_cols = HPC * (D + 2)
    has_bias = bool(np.any(qkv_b))
    c_in = C + 1 if has_bias else C

    steps, sch_steps = pair_schedule(HPC, N // 128)
    nsteps = len(steps)
    act_steps = tuple(s for s in range(nsteps) if s not in sch_steps)
    nact, nsch = len(act_steps), len(sch_steps)

    # per-batch transposed activations
    xts = []
    for b in range(B):
        xb = x[b].T  # [C, N]
        if has_bias:
            xb = np.concatenate([xb, np.ones((1, N), np.float32)], axis=0)
        xts.append(np.ascontiguousarray(xb))

    maps = []
    for core in range(NCORES):
        b = core // 4
        heads = [HPC * (core % 4) + i for i in range(HPC)]

        wqk = np.zeros((c_in, wqk_cols), np.float32)
        wv = np.zeros((c_in, wv_cols), np.float32)
        pwm = np.zeros((HPC * D, C), np.float32)
        for i, h in enumerate(heads):
            base = 128 * (2 * (i // 2)) + 64 * (i % 2)
            wqk[:C, base:base + 64] = qkv_w[D * h:D * h + D, :].T * SCALE
            kbase = 128 * (2 * (i // 2) + 1) + 64 * (i % 2)
            wqk[:C, kbase:kbase + 64] = qkv_w[C + D * h:C + D * h + D, :].T
            wv[:C, (D + 2) * i:(D + 2) * i + D] = qkv_w[2 * C + D * h:2 * C + D * h + D, :].T
            if has_bias:
                wqk[C, base:base + 64] = qkv_b[D * h:D * h + D] * SCALE
                wqk[C, kbase:kbase + 64] = qkv_b[C + D * h:C + D * h + D]
                wv[C, (D + 2) * i:(D + 2) * i + D] = qkv_b[2 * C + D * h:2 * C + D * h + D]
            pwm[64 * i:64 * i + 64, :] = proj_w[:, D * h:D * h + D].T
        # duplicate the odd head's q/k weights into the pad half-columns
        # (rows 64-127 of qk chunks 2/3) for row-tiled ST pairing
        if HPC % 2 == 1:
            i = HPC - 1
            base = 128 * (2 * (i // 2))
            kbase = base + 128
            wqk[:, base + 64:base + 128] = wqk[:, base:base + 64]
            wqk[:, kbase + 64:kbase + 128] = wqk[:, kbase:kbase + 64]

        mb = (mask[b, 0] != 0)                       # [N, N] bool
        bTs = [rel_bias[h].T for h in heads]          # [k, q] per head
        mT = mb.T
        ebB = np.empty((NQTR, nact, 128, 2 * QTR), ml_dtypes.bfloat16)
        ebT = np.empty((NQTR, nsch, 128, 2 * QTR), np.int16)
        for qtr in range(NQTR):
            cols = slice(qtr * QTR, qtr * QTR + QTR)
            for s, ((hA, jA), (hB, jB)) in enumerate(steps):
                blocks = []
                for (h, j) in ((hA, jA), (hB, jB)):
                    rows = slice(128 * j, 128 * j + 128)
                    blocks.append((bTs[h][rows, cols], mT[rows, cols]))
                if s in sch_steps:
                    si = sch_steps.index(s)
                    for k, (bT, mTk) in enumerate(blocks):
                        t = np.rint(A16 * bT + (B16 - C16))
                        ebT[qtr, si, :, k * QTR:(k + 1) * QTR] = np.where(
                            mTk, t, float(T_MASKED)).astype(np.int16)
                else:
                    ai = act_steps.index(s)
                    for k, (bT, mTk) in enumerate(blocks):
                        ebB[qtr, ai, :, k * QTR:(k + 1) * QTR] = (
                            np.exp(bT) * mTk).astype(ml_dtypes.bfloat16)

        maps.append({
            "xt": xts[b].astype(ml_dtypes.bfloat16),
            "wqk": wqk.astype(ml_dtypes.bfloat16),
            "wv": wv.astype(ml_dtypes.bfloat16),
            "ebb": ebB,
            "ebi": ebT,
            "pw": pwm.astype(ml_dtypes.bfloat16),
        })
    return maps, has_bias


def kernel(x, mask, qkv_w, qkv_b, rel_bias, proj_w, proj_b):
    global LAST_RESULTS
    maps, has_bias = make_in_maps(x, mask, qkv_w, qkv_b, rel_bias, proj_w)
    nc = _get_program(c_in=C + 1 if has_bias else C)

    trace = bool(os.environ.get("KERNEL_TRACE"))
    try:
        res = run_bass_kernel_spmd(
            nc, maps, list(range(NCORES)),
            trace=trace,
            trace_cores=list(range(NCORES)) if trace else None,
        )
    except Exception:
        if not trace:
            raise
        os.environ["BASS_NEVER_TRACE"] = "1"
        res = run_bass_kernel_spmd(nc, maps, list(range(NCORES)), trace=False)
    LAST_RESULTS = res

    proj_b = np.asarray(proj_b, dtype=np.float32)
    out = np.empty((B, N, C), np.float32)
    for b in range(B):
        acc = res.results[4 * b]["yt"].astype(np.float32)
        for c in range(4 * b + 1, 4 * b + 4):
            acc = acc + res.results[c]["yt"]
        out[b] = acc.T + proj_b[None, :]
    return out


# revision 15
# speedup vs baseline: 1.0289x; 1.0289x over previous
"""Trainium2 Bass kernel for a 12-head MHA layer with relative position bias
and a 0/1 attention mask (B=2, N=2048, C=768, H=12, d=64), sharded over 8
NeuronCores (batch x head-group parallel: core c handles batch c//4 and heads
3*(c%4) .. 3*(c%4)+2).

v3: phase D runs PAIRED k-tile steps: two ST matmuls execute concurrently on
row-groups (0,0)/(64,0) of the PE array (head h0/h2even lives at partitions
0-63, h1/h2odd at 64-127; the odd head's q/k weights are duplicated into the
pad half-columns so both row groups stay balanced 24:24). Each paired step
produces one [128, 1024] PSUM tile = [ST_A 512 | ST_B 512] over a q-quarter,
consumed by either:
  - ACT path: exp -> bf16, then DVE mult with a bf16 exp(bias)*mask pair tile
  - fused DVE path (1/4 of steps): e1_bits = int16(S*A16 + T) in one
    scalar_tensor_tensor, T = round(A16*bias + (B16-C)) with mask folded as a
    -25000 sentinel (bitcasts to ~-5e-9); e1 = bitcast_bf16(e1_bits)
The eb stream is host-packed in (quarter, step) schedule order so each step
is one DMA. PV accumulates per (head, quarter) into 1-bank [66,512] tiles
(row 64 = softmax denominator via the ones-column in v').

Junk warm-keeper matmuls cover the E-phase ln/exp window so the final
projection runs with the PE clock still hot.
"""

import os
import numpy as np
import ml_dtypes

import concourse.bass as bass
import concourse.tile as tile
from concourse.tile import add_dep_helper
from concourse import bacc, mybir
from concourse.alu_op_type import AluOpType
from concourse.bass_utils import run_bass_kernel_spmd

AF = mybir.ActivationFunctionType
DT = mybir.dt
F32R = mybir.dt.float32r

B, N, C, H, D = 2, 2048, 768, 12, 64
HPC = H // 4          # heads per core (8 cores = 2 batches x 4 head-groups)
NCORES = 8
SCALE = float(D) ** -0.5

# ---- fused integer-exp (Schraudolph, int16/bf16-bitcast form) ----
A16 = (1 << 7) / np.log(2.0)          # 184.66496...
B16 = 127 << 7                        # 16256
C16 = 5.1                             # spline-center correction (HW-tuned)
T_MASKED = -25000                     # int16 sentinel -> bf16 ~ -5e-9

QTR = 512                             # q-quarter width
NQTR = N // QTR

LAST_RESULTS = None   # BassKernelResults of the most recent kernel() call


def pair_schedule(hpc=HPC, nt=16):
    """Paired step schedule: Q0 runs on PE row-group 0 (partition base 0),
    Q1 on row-group 64. Heads i%2==0 live at base 0, i%2==1 at base 64; the
    odd head (h2) is split even-j/odd-j across the bases via weight dup.
    Returns (steps, sch_steps): steps[s] = ((headA, jA), (headB, jB))."""
    assert hpc == 3
    q0 = [(0, j) for j in range(nt)] + [(2, j) for j in range(0, nt, 2)]
    q1 = [(1, j) for j in range(nt)] + [(2, j) for j in range(1, nt, 2)]
    steps = list(zip(q0, q1))
    sch = tuple(s for s in range(len(steps)) if s % 4 == 2)  # 6/24 = 1/4
    return steps, sch


def build_program(n=N, c_in=C, hpc=HPC, d=D, c_out=C):
    """Build the per-core Bass/Tile program. Same program runs on all cores
    (SPMD); per-core data differs via in_maps."""
    nt = n // 128                       # number of 128-row k-tiles
    qch = _q_chunks(n)
    ck = (c_in + 127) // 128            # contraction chunks over c_in
    n_qk_chunks = 2 * ((hpc + 1) // 2)  # 4 for hpc=3
    wqk_cols = 128 * n_qk_chunks
    wv_cols = hpc * (d + 2)             # [v_i | ones | pad] per head
    mo = c_out // 128                   # proj output row chunks

    steps, sch_steps = pair_schedule(hpc, nt)
    nsteps = len(steps)
    act_steps = tuple(s for s in range(nsteps) if s not in sch_steps)
    nact, nsch = len(act_steps), len(sch_steps)

    def pc(kc):
        return min(128, c_in - 128 * kc)

    nc = bacc.Bacc("TRN2", target_bir_lowering=False, debug=False)
    xt = nc.dram_tensor("xt", [c_in, n], DT.bfloat16, kind="ExternalInput").ap()
    wqk = nc.dram_tensor("wqk", [c_in, wqk_cols], DT.bfloat16, kind="ExternalInput").ap()
    wv = nc.dram_tensor("wv", [c_in, wv_cols], DT.bfloat16, kind="ExternalInput").ap()
    ebb = nc.dram_tensor("ebb", [NQTR, nact, 128, 2 * QTR], DT.bfloat16,
                         kind="ExternalInput").ap()
    ebi = nc.dram_tensor("ebi", [NQTR, nsch, 128, 2 * QTR], DT.int16,
                         kind="ExternalInput").ap()
    pw = nc.dram_tensor("pw", [hpc * d, c_out], DT.bfloat16, kind="ExternalInput").ap()
    yt = nc.dram_tensor("yt", [c_out, n], DT.float32, kind="ExternalOutput").ap()

    with tile.TileContext(nc) as tc:
        # ---- pools (stack allocator: xts/weights released before attn) ----
        persist = tc.alloc_tile_pool(name="persist", bufs=1)
        qkvout = tc.alloc_tile_pool(name="qkvout", bufs=1)
        loadp = tc.alloc_tile_pool(name="loadp", bufs=1)
        ps_qkv = tc.alloc_tile_pool(name="ps_qkv", bufs=4, space="PSUM")

        pw_s = persist.tile([64, hpc, c_out], DT.bfloat16, tag="pw")
        ones_s = persist.tile([1, 128], DT.float32, tag="ones")
        nc.vector.memset(ones_s, 1.0)
        ones3 = persist.tile([128, hpc], DT.float32, tag="ones3")
        nc.vector.memset(ones3, 1.0)
        ones_r = persist.tile([1, 128], F32R, tag="ones_r")
        nc.vector.tensor_copy(ones_r, ones_s)
        for i in range(hpc):
            nc.gpsimd.dma_start(out=pw_s[:, i, :], in_=pw[64 * i:64 * i + 64, :])

        qk_s = qkvout.tile([128, n_qk_chunks, n], DT.bfloat16, tag="qk")
        v_s = qkvout.tile([128, nt, wv_cols], DT.bfloat16, tag="v")

        xts = loadp.tile([128, ck, n], DT.bfloat16, tag="xts")
        wqk_s = loadp.tile([128, ck, wqk_cols], DT.bfloat16, tag="wqk")
        wv_s = loadp.tile([128, ck, wv_cols], DT.bfloat16, tag="wv")
        for kc in range(ck):
            p = pc(kc)
            nc.gpsimd.dma_start(out=xts[:p, kc, :], in_=xt[128 * kc:128 * kc + p, :])
            nc.gpsimd.dma_start(out=wqk_s[:p, kc, :], in_=wqk[128 * kc:128 * kc + p, :])
            nc.gpsimd.dma_start(out=wv_s[:p, kc, :], in_=wv[128 * kc:128 * kc + p, :])

        # ---- phase B: qkT = wqk.T @ xT  -> qk_s ----
        def emit_qk_chunk(m):
            for (fo, fs) in qch:
                ps = ps_qkv.tile([128, 512], DT.float32, tag="psqkv", name=f"psB{m}")
                for kc in range(ck):
                    p = pc(kc)
                    nc.tensor.matmul(
                        ps[:, :fs],
                        lhsT=wqk_s[:p, kc, 128 * m:128 * m + 128],
                        rhs=xts[:p, kc, fo:fo + fs],
                        start=(kc == 0), stop=(kc == ck - 1),
                    )
                nc.vector.tensor_copy(qk_s[:, m, fo:fo + fs], ps[:, :fs])

        for m in range(min(2, n_qk_chunks)):
            emit_qk_chunk(m)

        # ---- phase C: v' = xT.T @ wv -> v_s (natural layout, k on partitions)
        for j in range(nt):
            ps = ps_qkv.tile([128, wv_cols], DT.float32, tag="psqkv")
            for kc in range(ck):
                p = pc(kc)
                nc.tensor.matmul(
                    ps,
                    lhsT=xts[:p, kc, 128 * j:128 * j + 128],
                    rhs=wv_s[:p, kc, :],
                    start=(kc == 0), stop=(kc == ck - 1),
                )
            nc.vector.tensor_copy(v_s[:, j, :], ps)
            nc.vector.tensor_copy(
                v_s[:, j, :].rearrange("p (h c) -> p h c", c=d + 2)[:, :, d],
                ones3)

        for m in range(min(2, n_qk_chunks), n_qk_chunks):
            emit_qk_chunk(m)

        loadp.release()   # free xts/wqk_s/wv_s space for attention pools
        ps_qkv.release()  # free PSUM banks for the attention pools

        # ---- attention pools ----
        ebpB = tc.alloc_tile_pool(name="ebpB", bufs=10)
        ebpI = tc.alloc_tile_pool(name="ebpI", bufs=5)
        e1pool = tc.alloc_tile_pool(name="e1pool", bufs=6)
        normp = tc.alloc_tile_pool(name="normp", bufs=1)
        ps_st = tc.alloc_tile_pool(name="ps_st", bufs=3, space="PSUM")
        ps_ot = tc.alloc_tile_pool(name="ps_ot", bufs=2, space="PSUM")

        osum_all = normp.tile([66, hpc, n], DT.bfloat16, tag="osum")
        osum = [osum_all[:, i, :] for i in range(hpc)]

        def unit_aps(head, j):
            """(q, kv, base) for a (head, j) unit in the paired layout."""
            if head < 2:
                base = 64 * head
                qv = qk_s[base:base + 64, 2 * (head // 2), :]
                kv = qk_s[base:base + 64, 2 * (head // 2) + 1, :]
            else:
                base = 64 * (j % 2)   # h2: even j at base 0, odd j at base 64
                qv = qk_s[base:base + 64, 2, :]
                kv = qk_s[base:base + 64, 3, :]
            return qv, kv

        # ---- phase D: paired attention steps over q-quarters ----
        last_exp = None
        for qtr in range(NQTR):
            qo = qtr * QTR
            ots = {}
            for s, ((hA, jA), (hB, jB)) in enumerate(steps):
                # eb pair tile for this step (one DMA, host-packed)
                if s in sch_steps:
                    si = sch_steps.index(s)
                    ebt = ebpI.tile([128, 2 * QTR], DT.int16, tag="ebT",
                                    name="ebT")
                    nc.gpsimd.dma_start(out=ebt, in_=ebi[qtr, si])
                else:
                    ai = act_steps.index(s)
                    ebt = ebpB.tile([128, 2 * QTR], DT.bfloat16, tag="ebB",
                                    name="ebB")
                    nc.gpsimd.dma_start(out=ebt, in_=ebb[qtr, ai])

                st = ps_st.tile([128, 2 * QTR], DT.float32, tag="st")
                for half, (h, j) in enumerate(((hA, jA), (hB, jB))):
                    qv, kv = unit_aps(h, j)
                    nc.tensor.matmul(
                        st[:, half * QTR:(half + 1) * QTR],
                        lhsT=kv[:, 128 * j:128 * j + 128],
                        rhs=qv[:, qo:qo + QTR],
                        start=True, stop=True,
                    )

                if s in sch_steps:
                    e1i = e1pool.tile([128, 2 * QTR], DT.int16, tag="e1i",
                                      name="e1i")
                    nc.vector.scalar_tensor_tensor(
                        e1i, st, float(A16), ebt,
                        AluOpType.mult, AluOpType.add)
                    e1x = e1i.bitcast(DT.bfloat16)
                else:
                    e0 = e1pool.tile([128, 2 * QTR], DT.bfloat16, tag="e0",
                                     name="e0")
                    last_exp = nc.scalar.activation(e0, st, AF.Exp)
                    e1 = e1pool.tile([128, 2 * QTR], DT.bfloat16, tag="e1",
                                     name="e1")
                    nc.vector.tensor_tensor(e1, e0, ebt, AluOpType.mult)
                    e1x = e1

                for half, (h, j) in enumerate(((hA, jA), (hB, jB))):
                    if h not in ots:
                        ots[h] = (ps_ot.tile([66, QTR], DT.float32, tag="ot",
                                             name=f"ot{h}"), [0])
                    ot, cnt = ots[h]
                    nc.tensor.matmul(
                        ot,
                        lhsT=v_s[:, j, (d + 2) * h:(d + 2) * h + d + 2],
                        rhs=e1x[:, half * QTR:(half + 1) * QTR],
                        start=(cnt[0] == 0), stop=(cnt[0] == nt - 1),
                    )
                    cnt[0] += 1
                    if cnt[0] == nt:
                        nc.vector.tensor_copy(osum[h][:, qo:qo + QTR], ot)
                        del ots[h]

        # ---- phase E: normalization (deferred; recip = exp(-ln(sum))) ----
        # single ln + exp over all heads' denominator rows at once
        lnr_all = normp.tile([1, hpc * n], DT.float32, tag="lnr", name="lnr")
        rrow_all = normp.tile([1, hpc * n], F32R, tag="rrow", name="rrow")
        ln_inst = nc.scalar.activation(
            lnr_all, osum_all[64:65, :, :].rearrange("p a b -> p (a b)"), AF.Ln)
        if last_exp is not None:
            add_dep_helper(ln_inst.ins, last_exp.ins, sync=False,
                           reason="act-table ordering")
        nc.scalar.activation(rrow_all, lnr_all, AF.Exp, scale=-1.0)
        rrow3 = rrow_all.rearrange("p (a b) -> p a b", b=n)

        for i in range(hpc):
            for h2 in range(2):
                ho = h2 * (n // 2)
                rps = ps_st.tile([64, n // 2], DT.float32, tag="st", name="rps")
                for (fo, fs) in _q_chunks(n // 2):
                    nc.tensor.matmul(
                        rps[:, fo:fo + fs],
                        lhsT=ones_r[0:1, 0:64],
                        rhs=rrow3[:, i, ho + fo:ho + fo + fs],
                        start=True, stop=True,
                    )
                # osum[0:64] *= 1/denom, fused from PSUM (in-place on in1)
                nc.vector.scalar_tensor_tensor(
                    osum[i][0:64, ho:ho + n // 2], rps, 1.0,
                    osum[i][0:64, ho:ho + n // 2],
                    AluOpType.mult, AluOpType.mult)

        ps_ot.release()
        ps_st.release()

        # ---- phase F: partial projection ytT = pw.T @ onrm ----
        ps_pj = tc.alloc_tile_pool(name="ps_pj", bufs=2, space="PSUM")
        ytp = tc.alloc_tile_pool(name="ytp", bufs=2)
        for m in range(mo):
            ps = ps_pj.tile([128, n], DT.float32, tag="pj")
            for (fo, fs) in qch:
                for i in range(hpc):
                    nc.tensor.matmul(
                        ps[:, fo:fo + fs],
                        lhsT=pw_s[:, i, 128 * m:128 * m + 128],
                        rhs=osum[i][0:64, fo:fo + fs],
                        start=(i == 0), stop=(i == hpc - 1),
                    )
            yts = ytp.tile([128, n], DT.float32, tag="yts")
            nc.scalar.copy(yts, ps)
            nc.sync.dma_start(out=yt[128 * m:128 * m + 128, :], in_=yts)

        ps_pj.release()
        ytp.release()
        normp.release()
        e1pool.release()
        ebpI.release()
        ebpB.release()
        qkvout.release()
        persist.release()

    nc.compile()
    return nc


def _q_chunks(n, c=512):
    out = []
    o = 0
    while o < n:
        sz = min(c, n - o)
        out.append((o, sz))
        o += sz
    return out


_PROG = {}


def _get_program(**kw):
    key = tuple(sorted(kw.items()))
    if key not in _PROG:
        _PROG[key] = build_program(**kw)
    return _PROG[key]


def make_in_maps(x, mask, qkv_w, qkv_b, rel_bias, proj_w):
    """Host-side shard + layout prep. Returns list of per-core input dicts."""
    x = np.asarray(x, dtype=np.float32)
    mask = np.asarray(mask)
    qkv_w = np.asarray(qkv_w, dtype=np.float32)
    qkv_b = np.asarray(qkv_b, dtype=np.float32)
    rel_bias = np.asarray(rel_bias, dtype=np.float32)
    proj_w = np.asarray(proj_w, dtype=np.float32)

    n_qk_chunks = 2 * ((HPC + 1) // 2)
    wqk_cols = 128 * n_qk_chunks
    wv_cols = HPC * (D + 2)
    has_bias = bool(np.any(qkv_b))
    c_in = C + 1 if has_bias else C

    steps, sch_steps = pair_schedule(HPC, N // 128)
    nsteps = len(steps)
    act_steps = tuple(s for s in range(nsteps) if s not in sch_steps)
    nact, nsch = len(act_steps), len(sch_steps)

    # per-batch transposed activations
    xts = []
    for b in range(B):
        xb = x[b].T  # [C, N]
        if has_bias:
            xb = np.concatenate([xb, np.ones((1, N), np.float32)], axis=0)
        xts.append(np.ascontiguousarray(xb))

    maps = []
    for core in range(NCORES):
        b = core // 4
        heads = [HPC * (core % 4) + i for i in range(HPC)]

        wqk = np.zeros((c_in, wqk_cols), np.float32)
        wv = np.zeros((c_in, wv_cols), np.float32)
        pwm = np.zeros((HPC * D, C), np.float32)
        for i, h in enumerate(heads):
            base = 128 * (2 * (i // 2)) + 64 * (i % 2)
            wqk[:C, base:base + 64] = qkv_w[D * h:D * h + D, :].T * SCALE
            kbase = 128 * (2 * (i // 2) + 1) + 64 * (i % 2)
            wqk[:C, kbase:kbase + 64] = qkv_w[C + D * h:C + D * h + D, :].T
            wv[:C, (D + 2) * i:(D + 2) * i + D] = qkv_w[2 * C + D * h:2 * C + D * h + D, :].T
            if has_bias:
                wqk[C, base:base + 64] = qkv_b[D * h:D * h + D] * SCALE
                wqk[C, kbase:kbase + 64] = qkv_b[C + D * h:C + D * h + D]
                wv[C, (D + 2) * i:(D + 2) * i + D] = qkv_b[2 * C + D * h:2 * C + D * h + D]
            pwm[64 * i:64 * i + 64, :] = proj_w[:, D * h:D * h + D].T
        # duplicate the odd head's q/k weights into the pad half-columns
        # (rows 64-127 of qk chunks 2/3) for row-tiled ST pairing
        if HPC % 2 == 1:
            i = HPC - 1
            base = 128 * (2 * (i // 2))
            kbase = base + 128
            wqk[:, base + 64:base + 128] = wqk[:, base:base + 64]
            wqk[:, kbase + 64:kbase + 128] = wqk[:, kbase:kbase + 64]

        mb = (mask[b, 0] != 0)                       # [N, N] bool
        bTs = [rel_bias[h].T for h in heads]          # [k, q] per head
        mT = mb.T
        ebB = np.empty((NQTR, nact, 128, 2 * QTR), ml_dtypes.bfloat16)
        ebT = np.empty((NQTR, nsch, 128, 2 * QTR), np.int16)
        for qtr in range(NQTR):
            cols = slice(qtr * QTR, qtr * QTR + QTR)
            for s, ((hA, jA), (hB, jB)) in enumerate(steps):
                blocks = []
                for (h, j) in ((hA, jA), (hB, jB)):
                    rows = slice(128 * j, 128 * j + 128)
                    blocks.append((bTs[h][rows, cols], mT[rows, cols]))
                if s in sch_steps:
                    si = sch_steps.index(s)
                    for k, (bT, mTk) in enumerate(blocks):
                        t = np.rint(A16 * bT + (B16 - C16))
                        ebT[qtr, si, :, k * QTR:(k + 1) * QTR] = np.where(
                            mTk, t, float(T_MASKED)).astype(np.int16)
                else:
                    ai = act_steps.index(s)
                    for k, (bT, mTk) in enumerate(blocks):
                        ebB[qtr, ai, :, k * QTR:(k + 1) * QTR] = (
                            np.exp(bT) * mTk).astype(ml_dtypes.bfloat16)

        maps.append({
            "xt": xts[b].astype(ml_dtypes.bfloat16),
            "wqk": wqk.astype(ml_dtypes.bfloat16),
            "wv": wv.astype(ml_dtypes.bfloat16),
            "ebb": ebB,
            "ebi": ebT,
            "pw": pwm.astype(ml_dtypes.bfloat16),
        })
    return maps, has_bias


def kernel(x, mask, qkv_w, qkv_b, rel_bias, proj_w, proj_b):
    global LAST_RESULTS
    maps, has_bias = make_in_maps(x, mask, qkv_w, qkv_b, rel_bias, proj_w)
    nc = _get_program(c_in=C + 1 if has_bias else C)

    trace = bool(os.environ.get("KERNEL_TRACE"))
    try:
        res = run_bass_kernel_spmd(
            nc, maps, list(range(NCORES)),
            trace=trace,
            trace_cores=list(range(NCORES)) if trace else None,
        )
    except Exception:
        if not trace:
            raise
        os.environ["BASS_NEVER_TRACE"] = "1"
        res = run_bass_kernel_spmd(nc, maps, list(range(NCORES)), trace=False)
    LAST_RESULTS = res

    proj_b = np.asarray(proj_b, dtype=np.float32)
    out = np.empty((B, N, C), np.float32)
    for b in range(B):
        acc = res.results[4 * b]["yt"].astype(np.float32)
        for c in range(4 * b + 1, 4 * b + 4):
            acc = acc + res.results[c]["yt"]
        out[b] = acc.T + proj_b[None, :]
    return out


# revision 22
# speedup vs baseline: 1.0483x; 1.0189x over previous
"""Trainium2 Bass kernel for a 12-head MHA layer with relative position bias
and a 0/1 attention mask (B=2, N=2048, C=768, H=12, d=64), sharded over 8
NeuronCores (batch x head-group parallel: core c handles batch c//4 and heads
3*(c%4) .. 3*(c%4)+2).

v3: phase D runs PAIRED k-tile steps: two ST matmuls execute concurrently on
row-groups (0,0)/(64,0) of the PE array (head h0/h2even lives at partitions
0-63, h1/h2odd at 64-127; the odd head's q/k weights are duplicated into the
pad half-columns so both row groups stay balanced 24:24). Each paired step
produces one [128, 1024] PSUM tile = [ST_A 512 | ST_B 512] over a q-quarter,
consumed by either:
  - ACT path: exp -> bf16, then DVE mult with a bf16 exp(bias)*mask pair tile
  - fused DVE path (1/4 of steps): e1_bits = int16(S*A16 + T) in one
    scalar_tensor_tensor, T = round(A16*bias + (B16-C)) with mask folded as a
    -25000 sentinel (bitcasts to ~-5e-9); e1 = bitcast_bf16(e1_bits)
The eb stream is host-packed in (quarter, step) schedule order so each step
is one DMA. PV accumulates per (head, quarter) into 1-bank [66,512] tiles
(row 64 = softmax denominator via the ones-column in v').

Junk warm-keeper matmuls cover the E-phase ln/exp window so the final
projection runs with the PE clock still hot.
"""

import os
import numpy as np
import ml_dtypes

import concourse.bass as bass
import concourse.tile as tile
from concourse.tile import add_dep_helper
from concourse import bacc, mybir
from concourse.alu_op_type import AluOpType
from concourse.bass_utils import run_bass_kernel_spmd

AF = mybir.ActivationFunctionType
DT = mybir.dt
F32R = mybir.dt.float32r

B, N, C, H, D = 2, 2048, 768, 12, 64
HPC = H // 4          # heads per core (8 cores = 2 batches x 4 head-groups)
NCORES = 8
SCALE = float(D) ** -0.5

# ---- fused integer-exp (Schraudolph, int16/bf16-bitcast form) ----
A16 = (1 << 7) / np.log(2.0)          # 184.66496...
B16 = 127 << 7                        # 16256
C16 = 5.1                             # spline-center correction (HW-tuned)
T_MASKED = -25000                     # int16 sentinel -> bf16 ~ -5e-9

QTR = 512                             # q-quarter width
NQTR = N // QTR

LAST_RESULTS = None   # BassKernelResults of the most recent kernel() call


def pair_schedule(hpc=HPC, nt=16, nqtr=4):
    """Paired step schedule: slot A runs on PE row-group 0 (partition base
    0), slot B on row-group 64. Heads i%2==0 live at base 0, i%2==1 at base
    64; the odd head (h2) is split even-j/odd-j across the bases via weight
    dup. D-I pairs (h0, h1) over all quarters, D-II pairs (h2even, h2odd) —
    so h2's projection chunks can be computed during D-I.
    Returns (steps, sch_steps): steps[s] = ((hA, jA, qtrA), (hB, jB, qtrB))."""
    assert hpc == 3
    steps = []
    for qtr in range(nqtr):
        for j in range(nt):
            steps.append(((0, j, qtr), (1, j, qtr)))
    for qtr in range(nqtr):
        for jj in range(nt // 2):
            steps.append(((2, 2 * jj, qtr), (2, 2 * jj + 1, qtr)))
    sch = tuple(s for s in range(len(steps)) if s % 4 == 2)  # 1/4 fused-DVE
    return steps, sch


def build_program(n=N, c_in=C, hpc=HPC, d=D, c_out=C):
    """Build the per-core Bass/Tile program. Same program runs on all cores
    (SPMD); per-core data differs via in_maps."""
    nt = n // 128                       # number of 128-row k-tiles
    qch = _q_chunks(n)
    ck = (c_in + 127) // 128            # contraction chunks over c_in
    n_qk_chunks = 2 * ((hpc + 1) // 2)  # 4 for hpc=3
    wqk_cols = 128 * n_qk_chunks
    wv_cols = hpc * (d + 2)             # [v_i | ones | pad] per head
    mo = c_out // 128                   # proj output row chunks

    steps, sch_steps = pair_schedule(hpc, nt)
    nsteps = len(steps)
    act_steps = tuple(s for s in range(nsteps) if s not in sch_steps)
    nact, nsch = len(act_steps), len(sch_steps)

    def pc(kc):
        return min(128, c_in - 128 * kc)

    nc = bacc.Bacc("TRN2", target_bir_lowering=False, debug=False)
    xt = nc.dram_tensor("xt", [c_in, n], DT.bfloat16, kind="ExternalInput").ap()
    wqk = nc.dram_tensor("wqk", [c_in, wqk_cols], DT.bfloat16, kind="ExternalInput").ap()
    wv = nc.dram_tensor("wv", [c_in, wv_cols], DT.bfloat16, kind="ExternalInput").ap()
    ebb = nc.dram_tensor("ebb", [nact, 128, 2 * QTR], DT.bfloat16,
                         kind="ExternalInput").ap()
    ebi = nc.dram_tensor("ebi", [nsch, 128, 2 * QTR], DT.int16,
                         kind="ExternalInput").ap()
    pw = nc.dram_tensor("pw", [hpc * d, c_out], DT.bfloat16, kind="ExternalInput").ap()
    yt = nc.dram_tensor("yt", [c_out, n], DT.float32, kind="ExternalOutput").ap()

    with tile.TileContext(nc) as tc:
        # ---- pools (stack allocator: xts/weights released before attn) ----
        persist = tc.alloc_tile_pool(name="persist", bufs=1)
        qkvout = tc.alloc_tile_pool(name="qkvout", bufs=1)
        loadp = tc.alloc_tile_pool(name="loadp", bufs=1)
        ps_qkv = tc.alloc_tile_pool(name="ps_qkv", bufs=4, space="PSUM")

        pw_s = persist.tile([64, hpc, c_out], DT.bfloat16, tag="pw")
        ones_s = persist.tile([1, 128], DT.float32, tag="ones")
        nc.vector.memset(ones_s, 1.0)
        ones3 = persist.tile([128, hpc], DT.float32, tag="ones3")
        nc.vector.memset(ones3, 1.0)
        ones_r = persist.tile([1, 128], F32R, tag="ones_r")
        nc.vector.tensor_copy(ones_r, ones_s)
        for i in range(hpc):
            nc.gpsimd.dma_start(out=pw_s[:, i, :], in_=pw[64 * i:64 * i + 64, :])

        qk_s = qkvout.tile([128, n_qk_chunks, n], DT.bfloat16, tag="qk")
        v_s = qkvout.tile([128, nt, wv_cols], DT.bfloat16, tag="v")

        xts = loadp.tile([128, ck, n], DT.bfloat16, tag="xts")
        wqk_s = loadp.tile([128, ck, wqk_cols], DT.bfloat16, tag="wqk")
        wv_s = loadp.tile([128, ck, wv_cols], DT.bfloat16, tag="wv")
        for kc in range(ck):
            p = pc(kc)
            nc.gpsimd.dma_start(out=xts[:p, kc, :], in_=xt[128 * kc:128 * kc + p, :])
            nc.gpsimd.dma_start(out=wqk_s[:p, kc, :], in_=wqk[128 * kc:128 * kc + p, :])
            nc.gpsimd.dma_start(out=wv_s[:p, kc, :], in_=wv[128 * kc:128 * kc + p, :])

        # ---- phase B/C emitters (interleaved into phase D's PE slack) ----
        # Early groups use ps_qkv (pre-attention); interleaved groups take a
        # rotation slot in the ps_st pool (tag-shared) so PSUM stays 8 banks.
        def emit_qk_group(m, fo, fs, pool, tag, width):
            ps = pool.tile([128, width], DT.float32, tag=tag, name=f"psB{m}")
            for kc in range(ck):
                p = pc(kc)
                nc.tensor.matmul(
                    ps[:, :fs],
                    lhsT=wqk_s[:p, kc, 128 * m:128 * m + 128],
                    rhs=xts[:p, kc, fo:fo + fs],
                    start=(kc == 0), stop=(kc == ck - 1),
                )
            nc.vector.tensor_copy(qk_s[:, m, fo:fo + fs], ps[:, :fs])

        def emit_v_group(j, pool, tag, width):
            ps = pool.tile([128, width], DT.float32, tag=tag, name=f"psC{j}")
            psv = ps[:, :wv_cols]
            for kc in range(ck):
                p = pc(kc)
                nc.tensor.matmul(
                    psv,
                    lhsT=xts[:p, kc, 128 * j:128 * j + 128],
                    rhs=wv_s[:p, kc, :],
                    start=(kc == 0), stop=(kc == ck - 1),
                )
            nc.vector.tensor_copy(v_s[:, j, :], psv)
            nc.vector.tensor_copy(
                v_s[:, j, :].rearrange("p (h c) -> p h c", c=d + 2)[:, :, d],
                ones3)

        # prologue: h0/h1 q+k projections and the first two v' tiles
        for m in (0, 1):
            for (fo, fs) in qch:
                emit_qk_group(m, fo, fs, ps_qkv, "psqkv", 512)
        for j in (0, 1):
            emit_v_group(j, ps_qkv, "psqkv", 512)

        ps_qkv.release()  # free PSUM banks for the attention pools
        # NOTE: loadp (xts/wqk/wv) stays alive through phase D — the
        # remaining B/C groups are interleaved into D's PE slack.

        # ---- attention pools ----
        ebpB = tc.alloc_tile_pool(name="ebpB", bufs=10)
        ebpI = tc.alloc_tile_pool(name="ebpI", bufs=5)
        e1pool = tc.alloc_tile_pool(name="e1pool", bufs=6)
        normp = tc.alloc_tile_pool(name="normp", bufs=1)
        ps_st = tc.alloc_tile_pool(name="ps_st", bufs=3, space="PSUM")
        ps_ot = tc.alloc_tile_pool(name="ps_ot", bufs=2, space="PSUM")
        ps_st._bc_tag = "st"

        osum_all = normp.tile([66, hpc, n], DT.bfloat16, tag="osum")
        osum = [osum_all[:, i, :] for i in range(hpc)]

        def unit_aps(head, j):
            """(q, kv) for a (head, j) unit in the paired layout."""
            if head < 2:
                base = 64 * head
                qv = qk_s[base:base + 64, 2 * (head // 2), :]
                kv = qk_s[base:base + 64, 2 * (head // 2) + 1, :]
            else:
                base = 64 * (j % 2)   # h2: even j at base 0, odd j at base 64
                qv = qk_s[base:base + 64, 2, :]
                kv = qk_s[base:base + 64, 3, :]
            return qv, kv

        # interleave plan: C(j=2..15) at steps 0..13, B m2/m3 groups after
        interleave = {}
        for s in range(14):
            interleave[s] = ("C", s + 2)
        bgroups = [(m, fo, fs) for m in (2, 3) for (fo, fs) in qch]
        for g, bg in enumerate(bgroups):
            interleave[16 + 3 * g] = ("B", bg)

        # ---- phase D: paired attention steps (D-I: h0/h1, D-II: h2) ----
        last_exp = None
        ots = {}
        for s, (ua, ub) in enumerate(steps):
            task = interleave.get(s)
            if task is not None:
                if task[0] == "C":
                    emit_v_group(task[1], ps_st, "st", 2 * QTR)
                else:
                    m, fo, fs = task[1]
                    emit_qk_group(m, fo, fs, ps_st, "st", 2 * QTR)

            # eb pair tile for this step (one DMA, host-packed)
            if s in sch_steps:
                si = sch_steps.index(s)
                ebt = ebpI.tile([128, 2 * QTR], DT.int16, tag="ebT",
                                name="ebT")
                nc.gpsimd.dma_start(out=ebt, in_=ebi[si])
            else:
                ai = act_steps.index(s)
                ebt = ebpB.tile([128, 2 * QTR], DT.bfloat16, tag="ebB",
                                name="ebB")
                nc.gpsimd.dma_start(out=ebt, in_=ebb[ai])

            st = ps_st.tile([128, 2 * QTR], DT.float32, tag="st")
            for half, (h, j, qtr) in enumerate((ua, ub)):
                qv, kv = unit_aps(h, j)
                nc.tensor.matmul(
                    st[:, half * QTR:(half + 1) * QTR],
                    lhsT=kv[:, 128 * j:128 * j + 128],
                    rhs=qv[:, qtr * QTR:qtr * QTR + QTR],
                    start=True, stop=True,
                )

            if s in sch_steps:
                e1i = e1pool.tile([128, 2 * QTR], DT.int16, tag="e1i",
                                  name="e1i")
                nc.vector.scalar_tensor_tensor(
                    e1i, st, float(A16), ebt,
                    AluOpType.mult, AluOpType.add)
                e1x = e1i.bitcast(DT.bfloat16)
            else:
                e0 = e1pool.tile([128, 2 * QTR], DT.bfloat16, tag="e0",
                                 name="e0")
                last_exp = nc.scalar.activation(e0, st, AF.Exp)
                e1 = e1pool.tile([128, 2 * QTR], DT.bfloat16, tag="e1",
                                 name="e1")
                nc.vector.tensor_tensor(e1, e0, ebt, AluOpType.mult)
                e1x = e1

            for half, (h, j, qtr) in enumerate((ua, ub)):
                key = (h, qtr)
                if key not in ots:
                    ots[key] = (ps_ot.tile([66, QTR], DT.float32, tag="ot",
                                           name=f"ot{h}_{qtr}"), [0])
                ot, cnt = ots[key]
                nc.tensor.matmul(
                    ot,
                    lhsT=v_s[:, j, (d + 2) * h:(d + 2) * h + d + 2],
                    rhs=e1x[:, half * QTR:(half + 1) * QTR],
                    start=(cnt[0] == 0), stop=(cnt[0] == nt - 1),
                )
                cnt[0] += 1
                if cnt[0] == nt:
                    nc.vector.tensor_copy(
                        osum[h][:, qtr * QTR:qtr * QTR + QTR], ot)
                    del ots[key]

        # ---- phase E: normalization (deferred; recip = exp(-ln(sum))) ----
        # single ln + exp over all heads' denominator rows at once
        lnr_all = normp.tile([1, hpc * n], DT.float32, tag="lnr", name="lnr")
        rrow_all = normp.tile([1, hpc * n], F32R, tag="rrow", name="rrow")
        ln_inst = nc.scalar.activation(
            lnr_all, osum_all[64:65, :, :].rearrange("p a b -> p (a b)"), AF.Ln)
        if last_exp is not None:
            add_dep_helper(ln_inst.ins, last_exp.ins, sync=False,
                           reason="act-table ordering")
        nc.scalar.activation(rrow_all, lnr_all, AF.Exp, scale=-1.0)
        rrow3 = rrow_all.rearrange("p (a b) -> p a b", b=n)

        for i in range(hpc):
            for h2 in range(2):
                ho = h2 * (n // 2)
                rps = ps_st.tile([64, n // 2], DT.float32, tag="st", name="rps")
                for (fo, fs) in _q_chunks(n // 2):
                    nc.tensor.matmul(
                        rps[:, fo:fo + fs],
                        lhsT=ones_r[0:1, 0:64],
                        rhs=rrow3[:, i, ho + fo:ho + fo + fs],
                        start=True, stop=True,
                    )
                # osum[0:64] *= 1/denom, fused from PSUM (in-place on in1)
                nc.vector.scalar_tensor_tensor(
                    osum[i][0:64, ho:ho + n // 2], rps, 1.0,
                    osum[i][0:64, ho:ho + n // 2],
                    AluOpType.mult, AluOpType.mult)

        ps_ot.release()
        ps_st.release()

        # ---- phase F: partial projection ytT = pw.T @ onrm ----
        ps_pj = tc.alloc_tile_pool(name="ps_pj", bufs=2, space="PSUM")
        ytp = tc.alloc_tile_pool(name="ytp", bufs=2)
        for m in range(mo):
            ps = ps_pj.tile([128, n], DT.float32, tag="pj")
            for (fo, fs) in qch:
                for i in range(hpc):
                    nc.tensor.matmul(
                        ps[:, fo:fo + fs],
                        lhsT=pw_s[:, i, 128 * m:128 * m + 128],
                        rhs=osum[i][0:64, fo:fo + fs],
                        start=(i == 0), stop=(i == hpc - 1),
                    )
            yts = ytp.tile([128, n], DT.float32, tag="yts")
            nc.scalar.copy(yts, ps)
            nc.sync.dma_start(out=yt[128 * m:128 * m + 128, :], in_=yts)

        ps_pj.release()
        ytp.release()
        normp.release()
        e1pool.release()
        ebpI.release()
        ebpB.release()
        loadp.release()
        qkvout.release()
        persist.release()

    nc.compile()
    return nc


def _q_chunks(n, c=512):
    out = []
    o = 0
    while o < n:
        sz = min(c, n - o)
        out.append((o, sz))
        o += sz
    return out


_PROG = {}


def _get_program(**kw):
    key = tuple(sorted(kw.items()))
    if key not in _PROG:
        _PROG[key] = build_program(**kw)
    return _PROG[key]


def make_in_maps(x, mask, qkv_w, qkv_b, rel_bias, proj_w):
    """Host-side shard + layout prep. Returns list of per-core input dicts."""
    x = np.asarray(x, dtype=np.float32)
    mask = np.asarray(mask)
    qkv_w = np.asarray(qkv_w, dtype=np.float32)
    qkv_b = np.asarray(qkv_b, dtype=np.float32)
    rel_bias = np.asarray(rel_bias, dtype=np.float32)
    proj_w = np.asarray(proj_w, dtype=np.float32)

    n_qk_chunks = 2 * ((HPC + 1) // 2)
    wqk_cols = 128 * n_qk_chunks
    wv_cols = HPC * (D + 2)
    has_bias = bool(np.any(qkv_b))
    c_in = C + 1 if has_bias else C

    steps, sch_steps = pair_schedule(HPC, N // 128)
    nsteps = len(steps)
    act_steps = tuple(s for s in range(nsteps) if s not in sch_steps)
    nact, nsch = len(act_steps), len(sch_steps)

    # per-batch transposed activations
    xts = []
    for b in range(B):
        xb = x[b].T  # [C, N]
        if has_bias:
            xb = np.concatenate([xb, np.ones((1, N), np.float32)], axis=0)
        xts.append(np.ascontiguousarray(xb))

    maps = []
    for core in range(NCORES):
        b = core // 4
        heads = [HPC * (core % 4) + i for i in range(HPC)]

        wqk = np.zeros((c_in, wqk_cols), np.float32)
        wv = np.zeros((c_in, wv_cols), np.float32)
        pwm = np.zeros((HPC * D, C), np.float32)
        for i, h in enumerate(heads):
            base = 128 * (2 * (i // 2)) + 64 * (i % 2)
            wqk[:C, base:base + 64] = qkv_w[D * h:D * h + D, :].T * SCALE
            kbase = 128 * (2 * (i // 2) + 1) + 64 * (i % 2)
            wqk[:C, kbase:kbase + 64] = qkv_w[C + D * h:C + D * h + D, :].T
            wv[:C, (D + 2) * i:(D + 2) * i + D] = qkv_w[2 * C + D * h:2 * C + D * h + D, :].T
            if has_bias:
                wqk[C, base:base + 64] = qkv_b[D * h:D * h + D] * SCALE
                wqk[C, kbase:kbase + 64] = qkv_b[C + D * h:C + D * h + D]
                wv[C, (D + 2) * i:(D + 2) * i + D] = qkv_b[2 * C + D * h:2 * C + D * h + D]
            pwm[64 * i:64 * i + 64, :] = proj_w[:, D * h:D * h + D].T
        # duplicate the odd head's q/k weights into the pad half-columns
        # (rows 64-127 of qk chunks 2/3) for row-tiled ST pairing
        if HPC % 2 == 1:
            i = HPC - 1
            base = 128 * (2 * (i // 2))
            kbase = base + 128
            wqk[:, base + 64:base + 128] = wqk[:, base:base + 64]
            wqk[:, kbase + 64:kbase + 128] = wqk[:, kbase:kbase + 64]

        mb = (mask[b, 0] != 0)                       # [N, N] bool
        bTs = [rel_bias[h].T for h in heads]          # [k, q] per head
        mT = mb.T
        ebB = np.empty((nact, 128, 2 * QTR), ml_dtypes.bfloat16)
        ebT = np.empty((nsch, 128, 2 * QTR), np.int16)
        for s, (ua, ub) in enumerate(steps):
            blocks = []
            for (h, j, qtr) in (ua, ub):
                rows = slice(128 * j, 128 * j + 128)
                cols = slice(qtr * QTR, qtr * QTR + QTR)
                blocks.append((bTs[h][rows, cols], mT[rows, cols]))
            if s in sch_steps:
                si = sch_steps.index(s)
                for k, (bT, mTk) in enumerate(blocks):
                    t = np.rint(A16 * bT + (B16 - C16))
                    ebT[si, :, k * QTR:(k + 1) * QTR] = np.where(
                        mTk, t, float(T_MASKED)).astype(np.int16)
            else:
                ai = act_steps.index(s)
                for k, (bT, mTk) in enumerate(blocks):
                    ebB[ai, :, k * QTR:(k + 1) * QTR] = (
                        np.exp(bT) * mTk).astype(ml_dtypes.bfloat16)

        maps.append({
            "xt": xts[b].astype(ml_dtypes.bfloat16),
            "wqk": wqk.astype(ml_dtypes.bfloat16),
            "wv": wv.astype(ml_dtypes.bfloat16),
            "ebb": ebB,
            "ebi": ebT,
            "pw": pwm.astype(ml_dtypes.bfloat16),
        })
    return maps, has_bias


def kernel(x, mask, qkv_w, qkv_b, rel_bias, proj_w, proj_b):
    global LAST_RESULTS
    maps, has_bias = make_in_maps(x, mask, qkv_w, qkv_b, rel_bias, proj_w)
    nc = _get_program(c_in=C + 1 if has_bias else C)

    trace = bool(os.environ.get("KERNEL_TRACE"))
    try:
        res = run_bass_kernel_spmd(
            nc, maps, list(range(NCORES)),
            trace=trace,
            trace_cores=list(range(NCORES)) if trace else None,
        )
    except Exception:
        if not trace:
            raise
        os.environ["BASS_NEVER_TRACE"] = "1"
        res = run_bass_kernel_spmd(nc, maps, list(range(NCORES)), trace=False)
    LAST_RESULTS = res

    proj_b = np.asarray(proj_b, dtype=np.float32)
    out = np.empty((B, N, C), np.float32)
    for b in range(B):
        acc = res.results[4 * b]["yt"].astype(np.float32)
        for c in range(4 * b + 1, 4 * b + 4):
            acc = acc + res.results[c]["yt"]
        out[b] = acc.T + proj_b[None, :]
    return out


# revision 27
# speedup vs baseline: 1.0573x; 1.0086x over previous
"""Trainium2 Bass kernel for a 12-head MHA layer with relative position bias
and a 0/1 attention mask (B=2, N=2048, C=768, H=12, d=64), sharded over 8
NeuronCores (batch x head-group parallel: core c handles batch c//4 and heads
3*(c%4) .. 3*(c%4)+2).

v3: phase D runs PAIRED k-tile steps: two ST matmuls execute concurrently on
row-groups (0,0)/(64,0) of the PE array (head h0/h2even lives at partitions
0-63, h1/h2odd at 64-127; the odd head's q/k weights are duplicated into the
pad half-columns so both row groups stay balanced 24:24). Each paired step
produces one [128, 1024] PSUM tile = [ST_A 512 | ST_B 512] over a q-quarter,
consumed by either:
  - ACT path: exp -> bf16, then DVE mult with a bf16 exp(bias)*mask pair tile
  - fused DVE path (1/4 of steps): e1_bits = int16(S*A16 + T) in one
    scalar_tensor_tensor, T = round(A16*bias + (B16-C)) with mask folded as a
    -25000 sentinel (bitcasts to ~-5e-9); e1 = bitcast_bf16(e1_bits)
The eb stream is host-packed in (quarter, step) schedule order so each step
is one DMA. PV accumulates per (head, quarter) into 1-bank [66,512] tiles
(row 64 = softmax denominator via the ones-column in v').

Junk warm-keeper matmuls cover the E-phase ln/exp window so the final
projection runs with the PE clock still hot.
"""

import os
import numpy as np
import ml_dtypes

import concourse.bass as bass
import concourse.tile as tile
from concourse.tile import add_dep_helper
from concourse import bacc, mybir
from concourse.alu_op_type import AluOpType
from concourse.bass_utils import run_bass_kernel_spmd

AF = mybir.ActivationFunctionType
DT = mybir.dt
F32R = mybir.dt.float32r

B, N, C, H, D = 2, 2048, 768, 12, 64
HPC = H // 4          # heads per core (8 cores = 2 batches x 4 head-groups)
NCORES = 8
SCALE = float(D) ** -0.5

# ---- fused integer-exp (Schraudolph, int16/bf16-bitcast form) ----
A16 = (1 << 7) / np.log(2.0)          # 184.66496...
B16 = 127 << 7                        # 16256
C16 = 5.1                             # spline-center correction (HW-tuned)
T_MASKED = -25000                     # int16 sentinel -> bf16 ~ -5e-9

QTR = 512                             # q-quarter width
NQTR = N // QTR

LAST_RESULTS = None   # BassKernelResults of the most recent kernel() call


def pair_schedule(hpc=HPC, nt=16, nqtr=4):
    """Paired step schedule: slot A runs on PE row-group 0 (partition base
    0), slot B on row-group 64. Heads i%2==0 live at base 0, i%2==1 at base
    64; the odd head (h2) is split even-j/odd-j across the bases via weight
    dup. D-I pairs (h0, h1) over all quarters, D-II pairs (h2even, h2odd) —
    so h2's projection chunks can be computed during D-I.
    Returns (steps, sch_steps): steps[s] = ((hA, jA, qtrA), (hB, jB, qtrB))."""
    assert hpc == 3
    steps = []
    for qtr in range(nqtr):
        for j in range(nt):
            steps.append(((0, j, qtr), (1, j, qtr)))
    for qtr in range(nqtr):
        for jj in range(nt // 2):
            steps.append(((2, 2 * jj, qtr), (2, 2 * jj + 1, qtr)))
    sch = tuple(s for s in range(len(steps)) if s % 4 == 2)  # 1/4 fused-DVE
    return steps, sch


def build_program(n=N, c_in=C, hpc=HPC, d=D, c_out=C):
    """Build the per-core Bass/Tile program. Same program runs on all cores
    (SPMD); per-core data differs via in_maps."""
    nt = n // 128                       # number of 128-row k-tiles
    qch = _q_chunks(n)
    ck = (c_in + 127) // 128            # contraction chunks over c_in
    n_qk_chunks = 2 * ((hpc + 1) // 2)  # 4 for hpc=3
    wqk_cols = 128 * n_qk_chunks
    wv_cols = hpc * (d + 2)             # [v_i | ones | pad] per head
    mo = c_out // 128                   # proj output row chunks

    steps, sch_steps = pair_schedule(hpc, nt)
    nsteps = len(steps)
    act_steps = tuple(s for s in range(nsteps) if s not in sch_steps)
    nact, nsch = len(act_steps), len(sch_steps)

    def pc(kc):
        return min(128, c_in - 128 * kc)

    nc = bacc.Bacc("TRN2", target_bir_lowering=False, debug=False)
    xt = nc.dram_tensor("xt", [c_in, n], DT.bfloat16, kind="ExternalInput").ap()
    wqk = nc.dram_tensor("wqk", [c_in, wqk_cols], DT.bfloat16, kind="ExternalInput").ap()
    wv = nc.dram_tensor("wv", [c_in, wv_cols], DT.bfloat16, kind="ExternalInput").ap()
    ebb = nc.dram_tensor("ebb", [nact, 128, 2 * QTR], DT.bfloat16,
                         kind="ExternalInput").ap()
    ebi = nc.dram_tensor("ebi", [nsch, 128, 2 * QTR], DT.int16,
                         kind="ExternalInput").ap()
    pw = nc.dram_tensor("pw", [hpc * d, c_out], DT.bfloat16, kind="ExternalInput").ap()
    yt = nc.dram_tensor("yt", [c_out, n], DT.float32, kind="ExternalOutput").ap()

    with tile.TileContext(nc) as tc:
        # ---- pools (stack allocator: xts/weights released before attn) ----
        persist = tc.alloc_tile_pool(name="persist", bufs=1)
        qkvout = tc.alloc_tile_pool(name="qkvout", bufs=1)
        loadp = tc.alloc_tile_pool(name="loadp", bufs=1)
        ps_qkv = tc.alloc_tile_pool(name="ps_qkv", bufs=4, space="PSUM")

        pw_s = persist.tile([64, hpc, c_out], DT.bfloat16, tag="pw")
        ones_s = persist.tile([1, 128], DT.float32, tag="ones")
        nc.vector.memset(ones_s, 1.0)
        ones3 = persist.tile([128, hpc], DT.float32, tag="ones3")
        nc.vector.memset(ones3, 1.0)
        ones_r = persist.tile([1, 128], F32R, tag="ones_r")
        nc.vector.tensor_copy(ones_r, ones_s)
        for i in range(hpc):
            nc.gpsimd.dma_start(out=pw_s[:, i, :], in_=pw[64 * i:64 * i + 64, :])

        qk_s = qkvout.tile([128, n_qk_chunks, n], DT.bfloat16, tag="qk")
        v_s = qkvout.tile([128, nt, wv_cols], DT.bfloat16, tag="v")

        xts = loadp.tile([128, ck, n], DT.bfloat16, tag="xts")
        wqk_s = loadp.tile([128, ck, wqk_cols], DT.bfloat16, tag="wqk")
        wv_s = loadp.tile([128, ck, wv_cols], DT.bfloat16, tag="wv")
        for kc in range(ck):
            p = pc(kc)
            nc.gpsimd.dma_start(out=xts[:p, kc, :], in_=xt[128 * kc:128 * kc + p, :])
            nc.gpsimd.dma_start(out=wqk_s[:p, kc, :], in_=wqk[128 * kc:128 * kc + p, :])
            nc.gpsimd.dma_start(out=wv_s[:p, kc, :], in_=wv[128 * kc:128 * kc + p, :])

        # ---- phase B/C emitters (interleaved into phase D's PE slack) ----
        # Early groups use ps_qkv (pre-attention); interleaved groups take a
        # rotation slot in the ps_st pool (tag-shared) so PSUM stays 8 banks.
        def emit_qk_group(m, fo, fs, pool, tag, width):
            ps = pool.tile([128, width], DT.float32, tag=tag, name=f"psB{m}")
            for kc in range(ck):
                p = pc(kc)
                nc.tensor.matmul(
                    ps[:, :fs],
                    lhsT=wqk_s[:p, kc, 128 * m:128 * m + 128],
                    rhs=xts[:p, kc, fo:fo + fs],
                    start=(kc == 0), stop=(kc == ck - 1),
                )
            nc.scalar.copy(qk_s[:, m, fo:fo + fs], ps[:, :fs])

        def emit_v_group(j, pool, tag, width):
            ps = pool.tile([128, width], DT.float32, tag=tag, name=f"psC{j}")
            psv = ps[:, :wv_cols]
            for kc in range(ck):
                p = pc(kc)
                nc.tensor.matmul(
                    psv,
                    lhsT=xts[:p, kc, 128 * j:128 * j + 128],
                    rhs=wv_s[:p, kc, :],
                    start=(kc == 0), stop=(kc == ck - 1),
                )
            nc.scalar.copy(v_s[:, j, :], psv)
            nc.vector.tensor_copy(
                v_s[:, j, :].rearrange("p (h c) -> p h c", c=d + 2)[:, :, d],
                ones3)

        # prologue: h0/h1 q+k projections and the first two v' tiles
        for m in (0, 1):
            for (fo, fs) in qch:
                emit_qk_group(m, fo, fs, ps_qkv, "psqkv", 512)
        for j in (0, 1):
            emit_v_group(j, ps_qkv, "psqkv", 512)

        ps_qkv.release()  # free PSUM banks for the attention pools
        # NOTE: loadp (xts/wqk/wv) stays alive through phase D — the
        # remaining B/C groups are interleaved into D's PE slack.

        # ---- attention pools ----
        ebpB = tc.alloc_tile_pool(name="ebpB", bufs=10)
        ebpI = tc.alloc_tile_pool(name="ebpI", bufs=5)
        e1pool = tc.alloc_tile_pool(name="e1pool", bufs=6)
        normp = tc.alloc_tile_pool(name="normp", bufs=1)
        ps_st = tc.alloc_tile_pool(name="ps_st", bufs=3, space="PSUM")
        ps_ot = tc.alloc_tile_pool(name="ps_ot", bufs=2, space="PSUM")
        ps_st._bc_tag = "st"

        osum_all = normp.tile([66, hpc, n], DT.bfloat16, tag="osum")
        osum = [osum_all[:, i, :] for i in range(hpc)]
        # denominator segments gathered to 12 partitions for a wide ln/exp
        den12 = normp.tile([hpc * NQTR, QTR], DT.bfloat16, tag="den12")

        def unit_aps(head, j):
            """(q, kv) for a (head, j) unit in the paired layout."""
            if head < 2:
                base = 64 * head
                qv = qk_s[base:base + 64, 2 * (head // 2), :]
                kv = qk_s[base:base + 64, 2 * (head // 2) + 1, :]
            else:
                base = 64 * (j % 2)   # h2: even j at base 0, odd j at base 64
                qv = qk_s[base:base + 64, 2, :]
                kv = qk_s[base:base + 64, 3, :]
            return qv, kv

        # interleave plan: C(j=2..15) at steps 0..13, B m2/m3 groups after
        interleave = {}
        for s in range(14):
            interleave[s] = ("C", s + 2)
        bgroups = [(m, fo, fs) for m in (2, 3) for (fo, fs) in qch]
        for g, bg in enumerate(bgroups):
            interleave[16 + 3 * g] = ("B", bg)

        # ---- phase D: paired attention steps (D-I: h0/h1, D-II: h2) ----
        last_exp = None
        ots = {}
        for s, (ua, ub) in enumerate(steps):
            task = interleave.get(s)
            if task is not None:
                if task[0] == "C":
                    emit_v_group(task[1], ps_st, "st", 2 * QTR)
                else:
                    m, fo, fs = task[1]
                    emit_qk_group(m, fo, fs, ps_st, "st", 2 * QTR)

            # eb pair tile for this step (one DMA, host-packed)
            if s in sch_steps:
                si = sch_steps.index(s)
                ebt = ebpI.tile([128, 2 * QTR], DT.int16, tag="ebT",
                                name="ebT")
                nc.gpsimd.dma_start(out=ebt, in_=ebi[si])
            else:
                ai = act_steps.index(s)
                ebt = ebpB.tile([128, 2 * QTR], DT.bfloat16, tag="ebB",
                                name="ebB")
                nc.gpsimd.dma_start(out=ebt, in_=ebb[ai])

            st = ps_st.tile([128, 2 * QTR], DT.float32, tag="st")
            for half, (h, j, qtr) in enumerate((ua, ub)):
                qv, kv = unit_aps(h, j)
                nc.tensor.matmul(
                    st[:, half * QTR:(half + 1) * QTR],
                    lhsT=kv[:, 128 * j:128 * j + 128],
                    rhs=qv[:, qtr * QTR:qtr * QTR + QTR],
                    start=True, stop=True,
                )

            if s in sch_steps:
                e1i = e1pool.tile([128, 2 * QTR], DT.int16, tag="e1i",
                                  name="e1i")
                nc.vector.scalar_tensor_tensor(
                    e1i, st, float(A16), ebt,
                    AluOpType.mult, AluOpType.add)
                e1x = e1i.bitcast(DT.bfloat16)
            else:
                e0 = e1pool.tile([128, 2 * QTR], DT.bfloat16, tag="e0",
                                 name="e0")
                last_exp = nc.scalar.activation(e0, st, AF.Exp)
                e1 = e1pool.tile([128, 2 * QTR], DT.bfloat16, tag="e1",
                                 name="e1")
                nc.vector.tensor_tensor(e1, e0, ebt, AluOpType.mult)
                e1x = e1

            for half, (h, j, qtr) in enumerate((ua, ub)):
                key = (h, qtr)
                if key not in ots:
                    ots[key] = (ps_ot.tile([66, QTR], DT.float32, tag="ot",
                                           name=f"ot{h}_{qtr}"), [0])
                ot, cnt = ots[key]
                nc.tensor.matmul(
                    ot,
                    lhsT=v_s[:, j, (d + 2) * h:(d + 2) * h + d + 2],
                    rhs=e1x[:, half * QTR:(half + 1) * QTR],
                    start=(cnt[0] == 0), stop=(cnt[0] == nt - 1),
                )
                cnt[0] += 1
                if cnt[0] == nt:
                    nc.vector.tensor_copy(
                        osum[h][:, qtr * QTR:qtr * QTR + QTR], ot)
                    r = h * NQTR + qtr
                    nc.sync.dma_start(
                        out=den12[r:r + 1, :],
                        in_=osum[h][64:65, qtr * QTR:qtr * QTR + QTR])
                    del ots[key]

        # ---- phase E: normalization (deferred; recip = exp(-ln(sum))) ----
        # 12-lane ln + exp over the gathered denominator segments, then
        # scatter the reciprocal rows back to partition 0 for the PE
        # broadcast (rps) matmuls
        lnr12 = normp.tile([hpc * NQTR, QTR], DT.float32, tag="lnr12",
                           name="lnr12")
        rrow12 = normp.tile([hpc * NQTR, QTR], F32R, tag="rrow12",
                            name="rrow12")
        rrow_all = normp.tile([1, hpc * n], F32R, tag="rrow", name="rrow")
        ln_inst = nc.scalar.activation(lnr12, den12, AF.Ln)
        if last_exp is not None:
            add_dep_helper(ln_inst.ins, last_exp.ins, sync=False,
                           reason="act-table ordering")
        nc.scalar.activation(rrow12, lnr12, AF.Exp, scale=-1.0)
        rrow3 = rrow_all.rearrange("p (a b) -> p a b", b=n)
        for i in range(hpc):
            for qtr in range(NQTR):
                r = i * NQTR + qtr
                nc.sync.dma_start(
                    out=rrow3[0:1, i, qtr * QTR:qtr * QTR + QTR],
                    in_=rrow12[r:r + 1, :])

        for i in range(hpc):
            for h2 in range(2):
                ho = h2 * (n // 2)
                rps = ps_st.tile([64, n // 2], DT.float32, tag="st", name="rps")
                for (fo, fs) in _q_chunks(n // 2):
                    nc.tensor.matmul(
                        rps[:, fo:fo + fs],
                        lhsT=ones_r[0:1, 0:64],
                        rhs=rrow3[:, i, ho + fo:ho + fo + fs],
                        start=True, stop=True,
                    )
                # osum[0:64] *= 1/denom, fused from PSUM (in-place on in1)
                nc.vector.scalar_tensor_tensor(
                    osum[i][0:64, ho:ho + n // 2], rps, 1.0,
                    osum[i][0:64, ho:ho + n // 2],
                    AluOpType.mult, AluOpType.mult)

        ps_ot.release()
        ps_st.release()

        # ---- phase F: partial projection ytT = pw.T @ onrm ----
        ps_pj = tc.alloc_tile_pool(name="ps_pj", bufs=2, space="PSUM")
        ytp = tc.alloc_tile_pool(name="ytp", bufs=2)
        for m in range(mo):
            ps = ps_pj.tile([128, n], DT.float32, tag="pj")
            for (fo, fs) in qch:
                for i in range(hpc):
                    nc.tensor.matmul(
                        ps[:, fo:fo + fs],
                        lhsT=pw_s[:, i, 128 * m:128 * m + 128],
                        rhs=osum[i][0:64, fo:fo + fs],
                        start=(i == 0), stop=(i == hpc - 1),
                    )
            yts = ytp.tile([128, n], DT.float32, tag="yts")
            nc.scalar.copy(yts, ps)
            nc.sync.dma_start(out=yt[128 * m:128 * m + 128, :], in_=yts)

        ps_pj.release()
        ytp.release()
        normp.release()
        e1pool.release()
        ebpI.release()
        ebpB.release()
        loadp.release()
        qkvout.release()
        persist.release()

    nc.compile()
    return nc


def _q_chunks(n, c=512):
    out = []
    o = 0
    while o < n:
        sz = min(c, n - o)
        out.append((o, sz))
        o += sz
    return out


_PROG = {}


def _get_program(**kw):
    key = tuple(sorted(kw.items()))
    if key not in _PROG:
        _PROG[key] = build_program(**kw)
    return _PROG[key]


def make_in_maps(x, mask, qkv_w, qkv_b, rel_bias, proj_w):
    """Host-side shard + layout prep. Returns list of per-core input dicts."""
    x = np.asarray(x, dtype=np.float32)
    mask = np.asarray(mask)
    qkv_w = np.asarray(qkv_w, dtype=np.float32)
    qkv_b = np.asarray(qkv_b, dtype=np.float32)
    rel_bias = np.asarray(rel_bias, dtype=np.float32)
    proj_w = np.asarray(proj_w, dtype=np.float32)

    n_qk_chunks = 2 * ((HPC + 1) // 2)
    wqk_cols = 128 * n_qk_chunks
    wv_cols = HPC * (D + 2)
    has_bias = bool(np.any(qkv_b))
    c_in = C + 1 if has_bias else C

    steps, sch_steps = pair_schedule(HPC, N // 128)
    nsteps = len(steps)
    act_steps = tuple(s for s in range(nsteps) if s not in sch_steps)
    nact, nsch = len(act_steps), len(sch_steps)

    # per-batch transposed activations
    xts = []
    for b in range(B):
        xb = x[b].T  # [C, N]
        if has_bias:
            xb = np.concatenate([xb, np.ones((1, N), np.float32)], axis=0)
        xts.append(np.ascontiguousarray(xb))

    maps = []
    for core in range(NCORES):
        b = core // 4
        heads = [HPC * (core % 4) + i for i in range(HPC)]

        wqk = np.zeros((c_in, wqk_cols), np.float32)
        wv = np.zeros((c_in, wv_cols), np.float32)
        pwm = np.zeros((HPC * D, C), np.float32)
        for i, h in enumerate(heads):
            base = 128 * (2 * (i // 2)) + 64 * (i % 2)
            wqk[:C, base:base + 64] = qkv_w[D * h:D * h + D, :].T * SCALE
            kbase = 128 * (2 * (i // 2) + 1) + 64 * (i % 2)
            wqk[:C, kbase:kbase + 64] = qkv_w[C + D * h:C + D * h + D, :].T
            wv[:C, (D + 2) * i:(D + 2) * i + D] = qkv_w[2 * C + D * h:2 * C + D * h + D, :].T
            if has_bias:
                wqk[C, base:base + 64] = qkv_b[D * h:D * h + D] * SCALE
                wqk[C, kbase:kbase + 64] = qkv_b[C + D * h:C + D * h + D]
                wv[C, (D + 2) * i:(D + 2) * i + D] = qkv_b[2 * C + D * h:2 * C + D * h + D]
            pwm[64 * i:64 * i + 64, :] = proj_w[:, D * h:D * h + D].T
        # duplicate the odd head's q/k weights into the pad half-columns
        # (rows 64-127 of qk chunks 2/3) for row-tiled ST pairing
        if HPC % 2 == 1:
            i = HPC - 1
            base = 128 * (2 * (i // 2))
            kbase = base + 128
            wqk[:, base + 64:base + 128] = wqk[:, base:base + 64]
            wqk[:, kbase + 64:kbase + 128] = wqk[:, kbase:kbase + 64]

        mb = (mask[b, 0] != 0)                       # [N, N] bool
        bTs = [rel_bias[h].T for h in heads]          # [k, q] per head
        mT = mb.T
        ebB = np.empty((nact, 128, 2 * QTR), ml_dtypes.bfloat16)
        ebT = np.empty((nsch, 128, 2 * QTR), np.int16)
        for s, (ua, ub) in enumerate(steps):
            blocks = []
            for (h, j, qtr) in (ua, ub):
                rows = slice(128 * j, 128 * j + 128)
                cols = slice(qtr * QTR, qtr * QTR + QTR)
                blocks.append((bTs[h][rows, cols], mT[rows, cols]))
            if s in sch_steps:
                si = sch_steps.index(s)
                for k, (bT, mTk) in enumerate(blocks):
                    t = np.rint(A16 * bT + (B16 - C16))
                    ebT[si, :, k * QTR:(k + 1) * QTR] = np.where(
                        mTk, t, float(T_MASKED)).astype(np.int16)
            else:
                ai = act_steps.index(s)
                for k, (bT, mTk) in enumerate(blocks):
                    ebB[ai, :, k * QTR:(k + 1) * QTR] = (
                        np.exp(bT) * mTk).astype(ml_dtypes.bfloat16)

        maps.append({
            "xt": xts[b].astype(ml_dtypes.bfloat16),
            "wqk": wqk.astype(ml_dtypes.bfloat16),
            "wv": wv.astype(ml_dtypes.bfloat16),
            "ebb": ebB,
            "ebi": ebT,
            "pw": pwm.astype(ml_dtypes.bfloat16),
        })
    return maps, has_bias


def kernel(x, mask, qkv_w, qkv_b, rel_bias, proj_w, proj_b):
    global LAST_RESULTS
    maps, has_bias = make_in_maps(x, mask, qkv_w, qkv_b, rel_bias, proj_w)
    nc = _get_program(c_in=C + 1 if has_bias else C)

    trace = bool(os.environ.get("KERNEL_TRACE"))
    try:
        res = run_bass_kernel_spmd(
            nc, maps, list(range(NCORES)),
            trace=trace,
            trace_cores=list(range(NCORES)) if trace else None,
        )
    except Exception:
        if not trace:
            raise
        os.environ["BASS_NEVER_TRACE"] = "1"
        res = run_bass_kernel_spmd(nc, maps, list(range(NCORES)), trace=False)
    LAST_RESULTS = res

    proj_b = np.asarray(proj_b, dtype=np.float32)
    out = np.empty((B, N, C), np.float32)
    for b in range(B):
        acc = res.results[4 * b]["yt"].astype(np.float32)
        for c in range(4 * b + 1, 4 * b + 4):
            acc = acc + res.results[c]["yt"]
        out[b] = acc.T + proj_b[None, :]
    return out


# revision 38
# speedup vs baseline: 1.0890x; 1.0299x over previous
"""Trainium2 Bass kernel for a 12-head MHA layer with relative position bias
and a 0/1 attention mask (B=2, N=2048, C=768, H=12, d=64), sharded over 8
NeuronCores (batch x head-group parallel: core c handles batch c//4 and heads
3*(c%4) .. 3*(c%4)+2).

v3: phase D runs PAIRED k-tile steps: two ST matmuls execute concurrently on
row-groups (0,0)/(64,0) of the PE array (head h0/h2even lives at partitions
0-63, h1/h2odd at 64-127; the odd head's q/k weights are duplicated into the
pad half-columns so both row groups stay balanced 24:24). Each paired step
produces one [128, 1024] PSUM tile = [ST_A 512 | ST_B 512] over a q-quarter,
consumed by either:
  - ACT path: exp -> bf16, then DVE mult with a bf16 exp(bias)*mask pair tile
  - fused DVE path (1/4 of steps): e1_bits = int16(S*A16 + T) in one
    scalar_tensor_tensor, T = round(A16*bias + (B16-C)) with mask folded as a
    -25000 sentinel (bitcasts to ~-5e-9); e1 = bitcast_bf16(e1_bits)
The eb stream is host-packed in (quarter, step) schedule order so each step
is one DMA. PV accumulates per (head, quarter) into 1-bank [66,512] tiles
(row 64 = softmax denominator via the ones-column in v').

Junk warm-keeper matmuls cover the E-phase ln/exp window so the final
projection runs with the PE clock still hot.
"""

import os
import numpy as np
import ml_dtypes

import concourse.bass as bass
import concourse.tile as tile
from concourse.tile import add_dep_helper
from concourse import bacc, mybir
from concourse.alu_op_type import AluOpType
from concourse.bass_utils import run_bass_kernel_spmd

AF = mybir.ActivationFunctionType
DT = mybir.dt
F32R = mybir.dt.float32r

B, N, C, H, D = 2, 2048, 768, 12, 64
HPC = H // 4          # heads per core (8 cores = 2 batches x 4 head-groups)
NCORES = 8
SCALE = float(D) ** -0.5

# ---- fused integer-exp (Schraudolph, int16/bf16-bitcast form) ----
A16 = (1 << 7) / np.log(2.0)          # 184.66496...
B16 = 127 << 7                        # 16256
C16 = 5.1                             # spline-center correction (HW-tuned)
T_MASKED = -25000                     # int16 sentinel -> bf16 ~ -5e-9

QTR = 512                             # q-quarter width
NQTR = N // QTR

LAST_RESULTS = None   # BassKernelResults of the most recent kernel() call


def pair_schedule(hpc=HPC, nt=16, nqtr=4):
    """Paired step schedule: slot A runs on PE row-group 0 (partition base
    0), slot B on row-group 64. Heads i%2==0 live at base 0, i%2==1 at base
    64; the odd head (h2) is split even-j/odd-j across the bases via weight
    dup. D-I pairs (h0, h1) over all quarters, D-II pairs (h2even, h2odd) —
    so h2's projection chunks can be computed during D-I.
    Returns (steps, sch_steps): steps[s] = ((hA, jA, qtrA), (hB, jB, qtrB))."""
    assert hpc == 3
    steps = []
    for qtr in range(nqtr):
        for j in range(nt):
            steps.append(((0, j, qtr), (1, j, qtr)))
    for qtr in range(nqtr):
        for jj in range(nt // 2):
            steps.append(((2, 2 * jj, qtr), (2, 2 * jj + 1, qtr)))
    sch = tuple(s for s in range(len(steps)) if s % 4 == 2)  # 1/4 fused-DVE
    return steps, sch


def build_program(n=N, c_in=C, hpc=HPC, d=D, c_out=C):
    """Build the per-core Bass/Tile program. Same program runs on all cores
    (SPMD); per-core data differs via in_maps."""
    nt = n // 128                       # number of 128-row k-tiles
    qch = _q_chunks(n)
    ck = (c_in + 127) // 128            # contraction chunks over c_in
    n_qk_chunks = 2 * ((hpc + 1) // 2)  # 4 for hpc=3
    wqk_cols = 128 * n_qk_chunks
    wv_cols = hpc * (d + 2)             # [v_i | ones | pad] per head
    mo = c_out // 128                   # proj output row chunks

    steps, sch_steps = pair_schedule(hpc, nt)
    nsteps = len(steps)
    act_steps = tuple(s for s in range(nsteps) if s not in sch_steps)
    nact, nsch = len(act_steps), len(sch_steps)

    def pc(kc):
        return min(128, c_in - 128 * kc)

    nc = bacc.Bacc("TRN2", target_bir_lowering=False, debug=False)
    xt = nc.dram_tensor("xt", [c_in, n], DT.bfloat16, kind="ExternalInput").ap()
    wqk = nc.dram_tensor("wqk", [c_in, wqk_cols], DT.bfloat16, kind="ExternalInput").ap()
    wv = nc.dram_tensor("wv", [c_in, wv_cols], DT.bfloat16, kind="ExternalInput").ap()
    ebb = nc.dram_tensor("ebb", [nact, 128, 2 * QTR], DT.bfloat16,
                         kind="ExternalInput").ap()
    ebi = nc.dram_tensor("ebi", [nsch, 128, 2 * QTR], DT.int16,
                         kind="ExternalInput").ap()
    pw = nc.dram_tensor("pw", [hpc * d, c_out], DT.bfloat16, kind="ExternalInput").ap()
    yt = nc.dram_tensor("yt", [c_out, n], DT.float32, kind="ExternalOutput").ap()

    with tile.TileContext(nc) as tc:
        # ---- pools (stack allocator: xts/weights released before attn) ----
        persist = tc.alloc_tile_pool(name="persist", bufs=1)
        qkvout = tc.alloc_tile_pool(name="qkvout", bufs=1)
        loadp = tc.alloc_tile_pool(name="loadp", bufs=1)
        ps_qkv = tc.alloc_tile_pool(name="ps_qkv", bufs=4, space="PSUM")

        pwA_s = persist.tile([128, c_out], DT.bfloat16, tag="pwA")
        pwB_s = persist.tile([64, c_out], DT.bfloat16, tag="pwB")
        ones_s = persist.tile([1, 128], DT.float32, tag="ones")
        nc.vector.memset(ones_s, 1.0)
        ones3 = persist.tile([128, hpc], DT.float32, tag="ones3")
        nc.vector.memset(ones3, 1.0)
        ones_r = persist.tile([1, 128], F32R, tag="ones_r")
        nc.vector.tensor_copy(ones_r, ones_s)
        nc.gpsimd.dma_start(out=pwA_s, in_=pw[0:128, :])
        nc.gpsimd.dma_start(out=pwB_s, in_=pw[128:hpc * 64, :])

        qk_s = qkvout.tile([128, n_qk_chunks, n], DT.bfloat16, tag="qk")
        v_s = qkvout.tile([128, nt, wv_cols], DT.bfloat16, tag="v")

        xts = loadp.tile([128, ck, n], DT.bfloat16, tag="xts")
        wqk_s = loadp.tile([128, ck, wqk_cols], DT.bfloat16, tag="wqk")
        wv_s = loadp.tile([128, ck, wv_cols], DT.bfloat16, tag="wv")
        for kc in range(ck):
            p = pc(kc)
            nc.gpsimd.dma_start(out=xts[:p, kc, :], in_=xt[128 * kc:128 * kc + p, :])
            nc.gpsimd.dma_start(out=wqk_s[:p, kc, :], in_=wqk[128 * kc:128 * kc + p, :])
            nc.gpsimd.dma_start(out=wv_s[:p, kc, :], in_=wv[128 * kc:128 * kc + p, :])

        # ---- phase B/C emitters (interleaved into phase D's PE slack) ----
        # Early groups use ps_qkv (pre-attention); interleaved groups take a
        # rotation slot in the ps_st pool (tag-shared) so PSUM stays 8 banks.
        def emit_qk_group(m, fo, fs, pool, tag, width):
            ps = pool.tile([128, width], DT.float32, tag=tag, name=f"psB{m}")
            for kc in range(ck):
                p = pc(kc)
                nc.tensor.matmul(
                    ps[:, :fs],
                    lhsT=wqk_s[:p, kc, 128 * m:128 * m + 128],
                    rhs=xts[:p, kc, fo:fo + fs],
                    start=(kc == 0), stop=(kc == ck - 1),
                )
            nc.scalar.copy(qk_s[:, m, fo:fo + fs], ps[:, :fs])

        def emit_v_group(j, pool, tag, width):
            ps = pool.tile([128, width], DT.float32, tag=tag, name=f"psC{j}")
            psv = ps[:, :wv_cols]
            for kc in range(ck):
                p = pc(kc)
                nc.tensor.matmul(
                    psv,
                    lhsT=xts[:p, kc, 128 * j:128 * j + 128],
                    rhs=wv_s[:p, kc, :],
                    start=(kc == 0), stop=(kc == ck - 1),
                )
            nc.scalar.copy(v_s[:, j, :], psv)
            nc.vector.tensor_copy(
                v_s[:, j, :].rearrange("p (h c) -> p h c", c=d + 2)[:, :, d],
                ones3)

        # prologue: h0/h1 q+k projections and the first two v' tiles
        for m in (0, 1):
            for (fo, fs) in qch:
                emit_qk_group(m, fo, fs, ps_qkv, "psqkv", 512)
        for j in (0, 1):
            emit_v_group(j, ps_qkv, "psqkv", 512)

        ps_qkv.release()  # free PSUM banks for the attention pools
        # NOTE: loadp (xts/wqk/wv) stays alive through phase D — the
        # remaining B/C groups are interleaved into D's PE slack.

        # ---- attention pools ----
        ebpB = tc.alloc_tile_pool(name="ebpB", bufs=10)
        ebpI = tc.alloc_tile_pool(name="ebpI", bufs=5)
        e1pool = tc.alloc_tile_pool(name="e1pool", bufs=6)
        normp = tc.alloc_tile_pool(name="normp", bufs=1)
        ps_st = tc.alloc_tile_pool(name="ps_st", bufs=3, space="PSUM")
        ps_ot = tc.alloc_tile_pool(name="ps_ot", bufs=2, space="PSUM")
        ps_st._bc_tag = "st"

        osum_all = normp.tile([66, hpc, n], DT.bfloat16, tag="osum")
        osum = [osum_all[:, i, :] for i in range(hpc)]
        # normalized h0/h1 repacked to partitions 0-63/64-127 before F
        osum01 = normp.tile([128, n], DT.bfloat16, tag="osum01")
        # denominator segments gathered to 12 partitions for a wide ln/exp
        den12 = normp.tile([hpc * NQTR, QTR], DT.bfloat16, tag="den12")

        def unit_aps(head, j):
            """(q, kv) for a (head, j) unit in the paired layout."""
            if head < 2:
                base = 64 * head
                qv = qk_s[base:base + 64, 2 * (head // 2), :]
                kv = qk_s[base:base + 64, 2 * (head // 2) + 1, :]
            else:
                base = 64 * (j % 2)   # h2: even j at base 0, odd j at base 64
                qv = qk_s[base:base + 64, 2, :]
                kv = qk_s[base:base + 64, 3, :]
            return qv, kv

        # interleave plan: C(j=2..15) at steps 0..13, B m2/m3 groups after
        interleave = {}
        for s in range(14):
            interleave[s] = ("C", s + 2)
        bgroups = [(m, fo, fs) for m in (2, 3) for (fo, fs) in qch]
        for g, bg in enumerate(bgroups):
            interleave[16 + 3 * g] = ("B", bg)

        # ---- phase D: paired attention steps (D-I: h0/h1, D-II: h2) ----
        last_exp = None
        ots = {}
        for s, (ua, ub) in enumerate(steps):
            task = interleave.get(s)
            if task is not None:
                if task[0] == "C":
                    emit_v_group(task[1], ps_st, "st", 2 * QTR)
                else:
                    m, fo, fs = task[1]
                    emit_qk_group(m, fo, fs, ps_st, "st", 2 * QTR)

            # eb pair tile for this step (one DMA, host-packed)
            if s in sch_steps:
                si = sch_steps.index(s)
                ebt = ebpI.tile([128, 2 * QTR], DT.int16, tag="ebT",
                                name="ebT")
                nc.gpsimd.dma_start(out=ebt, in_=ebi[si])
            else:
                ai = act_steps.index(s)
                ebt = ebpB.tile([128, 2 * QTR], DT.bfloat16, tag="ebB",
                                name="ebB")
                nc.gpsimd.dma_start(out=ebt, in_=ebb[ai])

            st = ps_st.tile([128, 2 * QTR], DT.float32, tag="st")
            for half, (h, j, qtr) in enumerate((ua, ub)):
                qv, kv = unit_aps(h, j)
                nc.tensor.matmul(
                    st[:, half * QTR:(half + 1) * QTR],
                    lhsT=kv[:, 128 * j:128 * j + 128],
                    rhs=qv[:, qtr * QTR:qtr * QTR + QTR],
                    start=True, stop=True,
                )

            if s in sch_steps:
                e1i = e1pool.tile([128, 2 * QTR], DT.int16, tag="e1i",
                                  name="e1i")
                nc.vector.scalar_tensor_tensor(
                    e1i, st, float(A16), ebt,
                    AluOpType.mult, AluOpType.add)
                e1x = e1i.bitcast(DT.bfloat16)
            else:
                e0 = e1pool.tile([128, 2 * QTR], DT.bfloat16, tag="e0",
                                 name="e0")
                last_exp = nc.scalar.activation(e0, st, AF.Exp)
                e1 = e1pool.tile([128, 2 * QTR], DT.bfloat16, tag="e1",
                                 name="e1")
                nc.vector.tensor_tensor(e1, e0, ebt, AluOpType.mult)
                e1x = e1

            for half, (h, j, qtr) in enumerate((ua, ub)):
                key = (h, qtr)
                if key not in ots:
                    ots[key] = (ps_ot.tile([66, QTR], DT.float32, tag="ot",
                                           name=f"ot{h}_{qtr}"), [0])
                ot, cnt = ots[key]
                nc.tensor.matmul(
                    ot,
                    lhsT=v_s[:, j, (d + 2) * h:(d + 2) * h + d + 2],
                    rhs=e1x[:, half * QTR:(half + 1) * QTR],
                    start=(cnt[0] == 0), stop=(cnt[0] == nt - 1),
                )
                cnt[0] += 1
                if cnt[0] == nt:
                    nc.vector.tensor_copy(
                        osum[h][:, qtr * QTR:qtr * QTR + QTR], ot)
                    r = h * NQTR + qtr
                    nc.sync.dma_start(
                        out=den12[r:r + 1, :],
                        in_=osum[h][64:65, qtr * QTR:qtr * QTR + QTR])
                    del ots[key]

        # ---- phase E: normalization (deferred; recip = exp(-ln(sum))) ----
        # 12-lane ln + exp over the gathered denominator segments, then
        # scatter the reciprocal rows back to partition 0 for the PE
        # broadcast (rps) matmuls
        lnr12 = normp.tile([hpc * NQTR, QTR], DT.float32, tag="lnr12",
                           name="lnr12")
        rrow12 = normp.tile([hpc * NQTR, QTR], F32R, tag="rrow12",
                            name="rrow12")
        rrow_all = normp.tile([1, hpc * n], F32R, tag="rrow", name="rrow")
        ln_inst = nc.scalar.activation(lnr12, den12, AF.Ln)
        if last_exp is not None:
            add_dep_helper(ln_inst.ins, last_exp.ins, sync=False,
                           reason="act-table ordering")
        nc.scalar.activation(rrow12, lnr12, AF.Exp, scale=-1.0)
        rrow3 = rrow_all.rearrange("p (a b) -> p a b", b=n)
        for i in range(hpc):
            for qtr in range(NQTR):
                r = i * NQTR + qtr
                nc.sync.dma_start(
                    out=rrow3[0:1, i, qtr * QTR:qtr * QTR + QTR],
                    in_=rrow12[r:r + 1, :])

        for i in range(hpc):
            for h2 in range(2):
                ho = h2 * (n // 2)
                rps = ps_st.tile([64, n // 2], DT.float32, tag="st", name="rps")
                for (fo, fs) in _q_chunks(n // 2):
                    nc.tensor.matmul(
                        rps[:, fo:fo + fs],
                        lhsT=ones_r[0:1, 0:64],
                        rhs=rrow3[:, i, ho + fo:ho + fo + fs],
                        start=True, stop=True,
                    )
                # osum[0:64] *= 1/denom, fused from PSUM (in-place on in1)
                nc.vector.scalar_tensor_tensor(
                    osum[i][0:64, ho:ho + n // 2], rps, 1.0,
                    osum[i][0:64, ho:ho + n // 2],
                    AluOpType.mult, AluOpType.mult)
            # repack normalized head into osum01 for the 128-contraction F
            if i == 0:
                nc.gpsimd.dma_start(out=osum01[0:64, :], in_=osum[0][0:64, :])
            elif i == 1:
                nc.gpsimd.dma_start(out=osum01[64:128, :], in_=osum[1][0:64, :])

        ps_ot.release()
        ps_st.release()

        # ---- phase F: partial projection ytT = pw.T @ onrm ----
        ps_pj = tc.alloc_tile_pool(name="ps_pj", bufs=2, space="PSUM")
        ytp = tc.alloc_tile_pool(name="ytp", bufs=2)
        for m in range(mo):
            ps = ps_pj.tile([128, n], DT.float32, tag="pj")
            for (fo, fs) in qch:
                nc.tensor.matmul(
                    ps[:, fo:fo + fs],
                    lhsT=pwA_s[:, 128 * m:128 * m + 128],
                    rhs=osum01[:, fo:fo + fs],
                    start=True, stop=False,
                )
                nc.tensor.matmul(
                    ps[:, fo:fo + fs],
                    lhsT=pwB_s[:, 128 * m:128 * m + 128],
                    rhs=osum[2][0:64, fo:fo + fs],
                    start=False, stop=True,
                )
            yts = ytp.tile([128, n], DT.float32, tag="yts")
            nc.scalar.copy(yts, ps)
            nc.sync.dma_start(out=yt[128 * m:128 * m + 128, :], in_=yts)

        ps_pj.release()
        ytp.release()
        normp.release()
        e1pool.release()
        ebpI.release()
        ebpB.release()
        loadp.release()
        qkvout.release()
        persist.release()

    nc.compile()
    return nc


def _q_chunks(n, c=512):
    out = []
    o = 0
    while o < n:
        sz = min(c, n - o)
        out.append((o, sz))
        o += sz
    return out


_PROG = {}


def _get_program(**kw):
    key = tuple(sorted(kw.items()))
    if key not in _PROG:
        _PROG[key] = build_program(**kw)
    return _PROG[key]


def make_in_maps(x, mask, qkv_w, qkv_b, rel_bias, proj_w):
    """Host-side shard + layout prep. Returns list of per-core input dicts."""
    x = np.asarray(x, dtype=np.float32)
    mask = np.asarray(mask)
    qkv_w = np.asarray(qkv_w, dtype=np.float32)
    qkv_b = np.asarray(qkv_b, dtype=np.float32)
    rel_bias = np.asarray(rel_bias, dtype=np.float32)
    proj_w = np.asarray(proj_w, dtype=np.float32)

    n_qk_chunks = 2 * ((HPC + 1) // 2)
    wqk_cols = 128 * n_qk_chunks
    wv_cols = HPC * (D + 2)
    has_bias = bool(np.any(qkv_b))
    c_in = C + 1 if has_bias else C

    steps, sch_steps = pair_schedule(HPC, N // 128)
    nsteps = len(steps)
    act_steps = tuple(s for s in range(nsteps) if s not in sch_steps)
    nact, nsch = len(act_steps), len(sch_steps)

    # per-batch transposed activations
    xts = []
    for b in range(B):
        xb = x[b].T  # [C, N]
        if has_bias:
            xb = np.concatenate([xb, np.ones((1, N), np.float32)], axis=0)
        xts.append(np.ascontiguousarray(xb))

    maps = []
    for core in range(NCORES):
        b = core // 4
        heads = [HPC * (core % 4) + i for i in range(HPC)]

        wqk = np.zeros((c_in, wqk_cols), np.float32)
        wv = np.zeros((c_in, wv_cols), np.float32)
        pwm = np.zeros((HPC * D, C), np.float32)
        for i, h in enumerate(heads):
            base = 128 * (2 * (i // 2)) + 64 * (i % 2)
            wqk[:C, base:base + 64] = qkv_w[D * h:D * h + D, :].T * SCALE
            kbase = 128 * (2 * (i // 2) + 1) + 64 * (i % 2)
            wqk[:C, kbase:kbase + 64] = qkv_w[C + D * h:C + D * h + D, :].T
            wv[:C, (D + 2) * i:(D + 2) * i + D] = qkv_w[2 * C + D * h:2 * C + D * h + D, :].T
            if has_bias:
                wqk[C, base:base + 64] = qkv_b[D * h:D * h + D] * SCALE
                wqk[C, kbase:kbase + 64] = qkv_b[C + D * h:C + D * h + D]
                wv[C, (D + 2) * i:(D + 2) * i + D] = qkv_b[2 * C + D * h:2 * C + D * h + D]
            pwm[64 * i:64 * i + 64, :] = proj_w[:, D * h:D * h + D].T
        # duplicate the odd head's q/k weights into the pad half-columns
        # (rows 64-127 of qk chunks 2/3) for row-tiled ST pairing
        if HPC % 2 == 1:
            i = HPC - 1
            base = 128 * (2 * (i // 2))
            kbase = base + 128
            wqk[:, base + 64:base + 128] = wqk[:, base:base + 64]
            wqk[:, kbase + 64:kbase + 128] = wqk[:, kbase:kbase + 64]

        mb = (mask[b, 0] != 0)                       # [N, N] bool
        bTs = [rel_bias[h].T for h in heads]          # [k, q] per head
        mT = mb.T
        ebB = np.empty((nact, 128, 2 * QTR), ml_dtypes.bfloat16)
        ebT = np.empty((nsch, 128, 2 * QTR), np.int16)
        for s, (ua, ub) in enumerate(steps):
            blocks = []
            for (h, j, qtr) in (ua, ub):
                rows = slice(128 * j, 128 * j + 128)
                cols = slice(qtr * QTR, qtr * QTR + QTR)
                blocks.append((bTs[h][rows, cols], mT[rows, cols]))
            if s in sch_steps:
                si = sch_steps.index(s)
                for k, (bT, mTk) in enumerate(blocks):
                    t = np.rint(A16 * bT + (B16 - C16))
                    ebT[si, :, k * QTR:(k + 1) * QTR] = np.where(
                        mTk, t, float(T_MASKED)).astype(np.int16)
            else:
                ai = act_steps.index(s)
                for k, (bT, mTk) in enumerate(blocks):
                    ebB[ai, :, k * QTR:(k + 1) * QTR] = (
                        np.exp(bT) * mTk).astype(ml_dtypes.bfloat16)

        maps.append({
            "xt": xts[b].astype(ml_dtypes.bfloat16),
            "wqk": wqk.astype(ml_dtypes.bfloat16),
            "wv": wv.astype(ml_dtypes.bfloat16),
            "ebb": ebB,
            "ebi": ebT,
            "pw": pwm.astype(ml_dtypes.bfloat16),
        })
    return maps, has_bias


def kernel(x, mask, qkv_w, qkv_b, rel_bias, proj_w, proj_b):
    global LAST_RESULTS
    maps, has_bias = make_in_maps(x, mask, qkv_w, qkv_b, rel_bias, proj_w)
    nc = _get_program(c_in=C + 1 if has_bias else C)

    trace = bool(os.environ.get("KERNEL_TRACE"))
    try:
        res = run_bass_kernel_spmd(
            nc, maps, list(range(NCORES)),
            trace=trace,
            trace_cores=list(range(NCORES)) if trace else None,
        )
    except Exception:
        if not trace:
            raise
        os.environ["BASS_NEVER_TRACE"] = "1"
        res = run_bass_kernel_spmd(nc, maps, list(range(NCORES)), trace=False)
    LAST_RESULTS = res

    proj_b = np.asarray(proj_b, dtype=np.float32)
    out = np.empty((B, N, C), np.float32)
    for b in range(B):
        acc = res.results[4 * b]["yt"].astype(np.float32)
        for c in range(4 * b + 1, 4 * b + 4):
            acc = acc + res.results[c]["yt"]
        out[b] = acc.T + proj_b[None, :]
    return out
